# revision 26
# baseline (speedup 1.0000x reference)
"""Bass/Trainium2 kernel for nn_DynamicRadiusChannelFusion.

Sharding: 8 cores; core j handles batch b=j//2, center half h=j%2 (1024
centers each); points/feats of batch b replicated to its two cores.

knn_idx must match the (neuron-executed, eager per-op) jax reference
bitwise: dist2 = max((a2+b2) - 2*inner, 0), inner = fp32 PE matmul,
selection = stable ascending sort (ties -> lower index). We compute
v2 = min(2*inner - (a2+b2), 0) (bitwise == -dist2), take per-512-chunk
top-8 (nc.vector.max), merge to the top-32 value multiset, then
full-width max_index reproduces the reference tie semantics exactly.
"""
import os
import numpy as np
SKIP_MLP = int(os.environ.get("SKIP_MLP", "0"))

B, N, M, C, OUT, K = 4, 16384, 2048, 128, 256, 32
ML = 1024           # centers per core
NT = ML // 128      # 8 m-tiles
EH = 2048           # edges per MLP half-tile

_cache = {}


def _build():
    import concourse.bacc as bacc
    import concourse.mybir as mybir
    from concourse.tile import TileContext

    f32 = mybir.dt.float32
    u32 = mybir.dt.uint32
    i16 = mybir.dt.int16
    i32 = mybir.dt.int32
    AF = mybir.ActivationFunctionType
    OP = mybir.AluOpType

    nc = bacc.Bacc("TRN2", target_bir_lowering=False, debug=False, num_devices=8)

    t_pointsT = nc.dram_tensor("pointsT", (3, N), f32, kind="ExternalInput")
    t_prows = nc.dram_tensor("prows", (128, N // 128 * 3), f32, kind="ExternalInput")
    t_feats = nc.dram_tensor("feats", (N, C), f32, kind="ExternalInput")
    t_centersT = nc.dram_tensor("centersT", (3, ML), f32, kind="ExternalInput")
    t_crows = nc.dram_tensor("crows", (128, NT * 3), f32, kind="ExternalInput")
    t_cidx = nc.dram_tensor("cidx16", (128, ML // 16), i16, kind="ExternalInput")
    t_w1 = nc.dram_tensor("w1", (C, 2 * C), f32, kind="ExternalInput")
    t_w2 = nc.dram_tensor("w2", (C, C), f32, kind="ExternalInput")
    t_w3 = nc.dram_tensor("w3", (C, OUT), f32, kind="ExternalInput")
    t_b1 = nc.dram_tensor("b1", (128, 1), f32, kind="ExternalInput")
    t_b2 = nc.dram_tensor("b2", (128, 1), f32, kind="ExternalInput")
    t_b3 = nc.dram_tensor("b3", (128, 2), f32, kind="ExternalInput")
    t_ident = nc.dram_tensor("ident", (128, 128), f32, kind="ExternalInput")
    t_e16 = nc.dram_tensor("e16", (16, 128), f32, kind="ExternalInput")
    t_sel = nc.dram_tensor("sel", (64, 4 * 512), f32, kind="ExternalInput")

    o_knn = nc.dram_tensor("o_knn", (ML, K), i32, kind="ExternalOutput")
    o_out = nc.dram_tensor("o_out", (ML, OUT), f32, kind="ExternalOutput")
    d_bsq = nc.dram_tensor("d_bsq", (1, N), f32, kind="Internal")
    o_gidx = nc.dram_tensor("o_gidx", (NT, 128, 256), i16, kind="ExternalOutput")

    with TileContext(nc) as tc:
        with tc.tile_pool(name="cons", bufs=1) as cons, \
             tc.tile_pool(name="big", bufs=1) as big, \
             tc.tile_pool(name="work", bufs=2) as work, \
             tc.tile_pool(name="fat", bufs=1) as fat, \
             tc.tile_pool(name="psA", bufs=1, space="PSUM") as psA, \
             tc.tile_pool(name="psB", bufs=1, space="PSUM") as psB, \
             tc.tile_pool(name="psC", bufs=1, space="PSUM") as psC:

            # ---------------- constants / inputs ----------------
            centersT = cons.tile([3, ML], f32)
            nc.sync.dma_start(centersT, t_centersT.ap())
            crows = cons.tile([128, NT * 3], f32)
            nc.sync.dma_start(crows, t_crows.ap())
            prows = fat.tile([128, N // 128 * 3], f32, tag="fatA")
            nc.sync.dma_start(prows, t_prows.ap())
            w1t = cons.tile([C, 2 * C], f32)
            nc.sync.dma_start(w1t, t_w1.ap())
            w1r = w1t.rearrange("p (h c) -> p h c", h=2)
            w2s = cons.tile([C, C], f32)
            nc.sync.dma_start(w2s, t_w2.ap())
            w3s = cons.tile([C, OUT], f32)
            nc.sync.dma_start(w3s, t_w3.ap())
            b1s = cons.tile([128, 1], f32)
            nc.sync.dma_start(b1s, t_b1.ap())
            b2s = cons.tile([128, 1], f32)
            nc.sync.dma_start(b2s, t_b2.ap())
            b3s = cons.tile([128, 2], f32)
            nc.sync.dma_start(b3s, t_b3.ap())
            ident = cons.tile([128, 128], f32)
            nc.sync.dma_start(ident, t_ident.ap())
            e16 = cons.tile([16, 128], f32)
            nc.sync.dma_start(e16, t_e16.ap())
            sel_t = cons.tile([64, 4 * 512], f32)
            nc.sync.dma_start(sel_t, t_sel.ap())
            sel = sel_t.rearrange("p (q n) -> p q n", q=4)
            cidx = cons.tile([128, ML // 16], i16)
            nc.sync.dma_start(cidx, t_cidx.ap())

            # ---- b_sq bitwise ((x*x + y*y) + z*z) ----
            pr3 = prows.rearrange("p (n c) -> p n c", c=3)
            sq = cons.tile([128, N // 128], f32)
            tq0 = work.tile([128, N // 128], f32, tag="tq0")
            tq1 = work.tile([128, N // 128], f32, tag="tq1")
            nc.vector.tensor_tensor(out=tq0, in0=pr3[:, :, 0], in1=pr3[:, :, 0], op=OP.mult)
            nc.vector.tensor_tensor(out=tq1, in0=pr3[:, :, 1], in1=pr3[:, :, 1], op=OP.mult)
            nc.vector.tensor_tensor(out=tq0, in0=tq0, in1=tq1, op=OP.add)
            nc.vector.tensor_tensor(out=tq1, in0=pr3[:, :, 2], in1=pr3[:, :, 2], op=OP.mult)
            nc.vector.tensor_tensor(out=sq, in0=tq0, in1=tq1, op=OP.add)
            nc.sync.dma_start(d_bsq.ap().rearrange("o (p n) -> p (o n)", p=128), sq)
            ones_col = cons.tile([1, 128], f32)
            nc.vector.memset(ones_col, 1.0)
            bbc = big.tile([128, N], f32)
            for c in range(N // 512):
                bqc = fat.tile([1, 512], f32, tag="fatB")
                nc.sync.dma_start(bqc, d_bsq.ap()[:, c * 512:(c + 1) * 512])
                pb = psB.tile([128, 512], f32, tag="mm512")
                nc.tensor.matmul(pb, ones_col, bqc, start=True, stop=True)
                nc.scalar.activation(bbc[:, c * 512:(c + 1) * 512], pb, AF.Copy,
                                     bias=0.0, scale=1.0)

            # ---- a_sq per center ----
            cr3 = crows.rearrange("p (t c) -> p t c", c=3)
            asq = cons.tile([128, NT], f32)
            ta0 = work.tile([128, NT], f32, tag="ta0")
            ta1 = work.tile([128, NT], f32, tag="ta1")
            nc.vector.tensor_tensor(out=ta0, in0=cr3[:, :, 0], in1=cr3[:, :, 0], op=OP.mult)
            nc.vector.tensor_tensor(out=ta1, in0=cr3[:, :, 1], in1=cr3[:, :, 1], op=OP.mult)
            nc.vector.tensor_tensor(out=ta0, in0=ta0, in1=ta1, op=OP.add)
            nc.vector.tensor_tensor(out=ta1, in0=cr3[:, :, 2], in1=cr3[:, :, 2], op=OP.mult)
            nc.vector.tensor_tensor(out=asq, in0=ta0, in1=ta1, op=OP.add)

            # ---- center feats gather + channel-major + Pa ----
            ctr_g = fat.tile([128, NT, C], f32, tag="fatB")      # m = t*128 + p
            nc.gpsimd.dma_gather(out_ap=ctr_g, in_ap=t_feats.ap(), idxs_ap=cidx,
                                 num_idxs=ML, num_idxs_reg=ML, elem_size=C)
            ctrT = cons.tile([C, ML], f32)
            for t in range(NT):
                pt = psC.tile([128, 128], f32, tag="tr")
                nc.tensor.transpose(pt, ctr_g[:, t, :], ident)
                nc.scalar.activation(ctrT[:, t * 128:(t + 1) * 128], pt, AF.Copy,
                                     bias=0.0, scale=1.0)
            # Pa = w1a^T @ ctrT : (128h, ML)
            Pa = fat.tile([128, ML], f32, tag="fatA")
            for q in range(ML // 512):
                pp = psB.tile([128, 512], f32, tag="mm512")
                nc.tensor.matmul(pp, w1r[:, 0, :], ctrT[:, q * 512:(q + 1) * 512],
                                 start=True, stop=True)
                nc.scalar.activation(Pa[:, q * 512:(q + 1) * 512], pp, AF.Copy,
                                     bias=0.0, scale=1.0)
            # PaT tiles: (m, h) layout per m-tile
            PaT = cons.tile([64, NT * 2, 128], f32)
            for t in range(NT):
                pt = psC.tile([128, 128], f32, tag="tr")
                nc.tensor.transpose(pt, Pa[:, t * 128:(t + 1) * 128], ident)
                nc.scalar.activation(PaT[:, t * 2 + 0, :], pt[0:64, :], AF.Copy, bias=0.0, scale=1.0)
                nc.scalar.activation(PaT[:, t * 2 + 1, :], pt[64:128, :], AF.Copy, bias=0.0, scale=1.0)

            fusedT = cons.tile([C, ML], f32)
            knn_all = cons.tile([128, NT, K], u32)

            # ================= per m-tile =================
            for t in range(NT):
                lhsT = centersT[:, t * 128:(t + 1) * 128]
                v2 = big.tile([128, N], f32, tag="v2")
                for c in range(N // 2048):
                    ptc = fat.tile([3, 2048], f32, tag="fatB")
                    nc.sync.dma_start(ptc, t_pointsT.ap()[:, c * 2048:(c + 1) * 2048])
                    pin = psA.tile([128, 2048], f32, tag="inner")
                    for q in range(4):
                        nc.tensor.matmul(pin[:, q * 512:(q + 1) * 512], lhsT,
                                         ptc[:, q * 512:(q + 1) * 512],
                                         start=True, stop=True)
                    sl = slice(c * 2048, (c + 1) * 2048)
                    tab = fat.tile([128, 2048], f32, tag="fatA")
                    nc.scalar.activation(tab, bbc[:, sl], AF.Relu,
                                         bias=asq[:, t:t + 1], scale=1.0)
                    nc.vector.scalar_tensor_tensor(v2[:, sl], pin, 2.0, tab,
                                                   op0=OP.mult, op1=OP.subtract)
                    nc.vector.tensor_scalar_min(v2[:, sl], v2[:, sl], 0.0)

                cand = work.tile([128, 256], f32, tag="cand")
                for c in range(32):
                    nc.vector.max(out=cand[:, c * 8:(c + 1) * 8],
                                  in_=v2[:, c * 512:(c + 1) * 512])
                scratch = work.tile([128, 256], f32, tag="scratch")
                nc.vector.tensor_copy(scratch, cand)
                fvals = work.tile([128, 32], f32, tag="fvals")
                for r in range(4):
                    nc.vector.max(out=fvals[:, r * 8:(r + 1) * 8], in_=scratch)
                    nc.vector.match_replace(out=scratch,
                                            in_to_replace=fvals[:, r * 8:(r + 1) * 8],
                                            in_values=scratch, imm_value=-3e38)
                knn_t = knn_all[:, t, :]
                for r in range(4):
                    nc.vector.max_index(knn_t[:, r * 8:(r + 1) * 8],
                                        fvals[:, r * 8:(r + 1) * 8], v2)
                    if r < 3:
                        nc.vector.match_replace(out=v2,
                                                in_to_replace=fvals[:, r * 8:(r + 1) * 8],
                                                in_values=v2, imm_value=-3e38)
                nc.sync.dma_start(o_knn.ap()[t * 128:(t + 1) * 128, :],
                                  knn_t.bitcast(i32))

                if SKIP_MLP == 1:
                    continue
                # ---- wrapped idx list (16, 256) -> replicate -> (128,256) --
                knn_f = work.tile([128, K], f32, tag="knnf")
                nc.vector.tensor_copy(knn_f, knn_t)
                ptk_a = psC.tile([16, 128], f32, tag="tr")
                nc.tensor.transpose(ptk_a, knn_f[:, 0:16], ident)
                ptk_b = psC.tile([16, 128], f32, tag="trb")
                nc.tensor.transpose(ptk_b, knn_f[:, 16:32], ident)
                wrap = work.tile([16, 256], f32, tag="wrap")
                wr = wrap.rearrange("p (m h) -> p m h", h=2)
                nc.vector.tensor_copy(wr[:, :, 0], ptk_a)
                nc.vector.tensor_copy(wr[:, :, 1], ptk_b)
                prep = psC.tile([128, 256], f32, tag="tr")
                nc.tensor.matmul(prep, e16, wrap, start=True, stop=True)
                gidx = work.tile([128, 256], i16, tag="gidx")
                nc.vector.tensor_copy(gidx, prep)
                if SKIP_MLP == 3:
                    nc.sync.dma_start(o_gidx.ap()[t], gidx)
                    continue

                # ---- gather neighbor feats + MLP in 2 half-tiles ----------
                for h in range(2):
                    gidx_h = work.tile([128, EH // 16], i16, tag="gidxh")
                    nc.vector.tensor_copy(gidx_h, gidx[:, h * 128:(h + 1) * 128])
                    xg = fat.tile([128, EH // 128, C], f32, tag="fatB")
                    for g4 in range(EH // 1024):
                        nc.gpsimd.dma_gather(
                            out_ap=xg[:, g4 * 8:(g4 + 1) * 8, :], in_ap=t_feats.ap(),
                            idxs_ap=gidx_h[:, g4 * 64:(g4 + 1) * 64],
                            num_idxs=1024, num_idxs_reg=1024, elem_size=C)
                    if SKIP_MLP == 4:
                        continue
                    xnT = fat.tile([C, EH], f32, tag="fatA")
                    for blk in range(EH // 128):
                        pt = psC.tile([128, 128], f32, tag="tr")
                        nc.tensor.transpose(pt, xg[:, blk, :], ident)
                        nc.scalar.activation(xnT[:, blk * 128:(blk + 1) * 128], pt,
                                             AF.Copy, bias=0.0, scale=1.0)
                    hsb = fat.tile([128, EH], f32, tag="fatB")
                    for q in range(EH // 512):
                        ph = psB.tile([128, 512], f32, tag="mm512")
                        nc.tensor.matmul(ph, w1r[:, 1, :],
                                         xnT[:, q * 512:(q + 1) * 512],
                                         start=True, stop=False)
                        nc.tensor.matmul(ph, PaT[:, t * 2 + h, :],
                                         sel[:, q, :],
                                         start=False, stop=True)
                        nc.scalar.activation(hsb[:, q * 512:(q + 1) * 512], ph,
                                             AF.Relu, bias=b1s, scale=1.0)
                    cw = big.tile([128, EH], f32, tag="v2")
                    for q in range(EH // 512):
                        pc = psB.tile([128, 512], f32, tag="mm512")
                        nc.tensor.matmul(pc, w2s,
                                         hsb[:, q * 512:(q + 1) * 512],
                                         start=True, stop=True)
                        nc.scalar.activation(cw[:, q * 512:(q + 1) * 512], pc,
                                             AF.Sigmoid, bias=b2s, scale=1.0)
                    prod = fat.tile([128, EH], f32, tag="fatB")
                    nc.vector.tensor_tensor(out=prod, in0=xnT, in1=cw, op=OP.mult)
                    wsum = work.tile([128, EH // K], f32, tag="wsum")
                    nc.vector.tensor_reduce(
                        wsum, prod.rearrange("p (m k) -> p m k", k=K),
                        axis=mybir.AxisListType.X, op=OP.add)
                    # fused = wsum/K + ctr feats (channel-major)
                    mlo = t * 128 + h * 64
                    nc.vector.scalar_tensor_tensor(
                        fusedT[:, mlo:mlo + 64], wsum, 1.0 / K,
                        ctrT[:, mlo:mlo + 64], op0=OP.mult, op1=OP.add)

            # ---- final layer: out = relu(w3^T @ fusedT + b3) -> (ML, OUT) --
            for t in range(NT if SKIP_MLP == 0 else 0):
                orow = work.tile([128, OUT], f32, tag="orow")
                for j in range(2):
                    po = psB.tile([128, 512], f32, tag="mm512")
                    nc.tensor.matmul(po[:, 0:128], w3s[:, j * 128:(j + 1) * 128],
                                     fusedT[:, t * 128:(t + 1) * 128],
                                     start=True, stop=True)
                    ot = work.tile([128, 128], f32, tag="otmp")
                    nc.scalar.activation(ot, po[:, 0:128],
                                         AF.Relu, bias=b3s[:, j:j + 1], scale=1.0)
                    pt = psC.tile([128, 128], f32, tag="tr")
                    nc.tensor.transpose(pt, ot, ident)
                    nc.scalar.activation(orow[:, j * 128:(j + 1) * 128], pt,
                                         AF.Copy, bias=0.0, scale=1.0)
                nc.sync.dma_start(o_out.ap()[t * 128:(t + 1) * 128, :], orow)

    nc.compile()
    return nc


def kernel(points, feats, center_idx, w1, b1, w2, b2, w3, b3):
    from concourse import bass_utils

    points = np.asarray(points); feats = np.asarray(feats)
    center_idx = np.asarray(center_idx)
    w1 = np.asarray(w1, dtype=np.float32); w2 = np.asarray(w2, dtype=np.float32)
    w3 = np.asarray(w3, dtype=np.float32)
    b1v = np.asarray(b1, dtype=np.float32); b2v = np.asarray(b2, dtype=np.float32)
    b3v = np.asarray(b3, dtype=np.float32)

    if "nc" not in _cache:
        _cache["nc"] = _build()
    nc = _cache["nc"]

    ident = np.eye(128, dtype=np.float32)
    e16 = np.tile(np.eye(16, dtype=np.float32), (1, 8)).reshape(16, 128)
    # e16[k, p] must be 1 iff p % 16 == k
    e16 = np.zeros((16, 128), dtype=np.float32)
    e16[np.arange(128) % 16, np.arange(128)] = 1.0
    sel = np.zeros((64, 4, 512), dtype=np.float32)
    for q in range(4):
        cols = np.arange(512)
        sel[q * 16 + cols // K, q, cols] = 1.0
    sel = sel.reshape(64, 4 * 512)
    b1r = b1v.reshape(128, 1)
    b2r = b2v.reshape(128, 1)
    b3r = b3v.reshape(2, 128).T.copy()   # b3r[p, j] = b3[j*128+p]

    in_maps = []
    for core in range(8):
        b = core // 2
        h = core % 2
        cid = center_idx[b, h * ML:(h + 1) * ML].astype(np.int64)
        P = points[b]
        ctr = P[cid]                                   # (ML, 3)
        # wrapped + replicated int16 idx layout for dma_gather
        cidx16 = np.zeros((128, ML // 16), dtype=np.int16)
        flat = cid.astype(np.int16)
        w = np.zeros((16, ML // 16), dtype=np.int16)
        w[np.arange(ML) % 16, np.arange(ML) // 16] = flat
        cidx16[:] = np.tile(w, (8, 1))
        in_maps.append({
            "pointsT": np.ascontiguousarray(P.T),
            "prows": P.reshape(128, N // 128 * 3).copy(),
            "feats": feats[b].copy(),
            "centersT": np.ascontiguousarray(ctr.T),
            "crows": ctr.reshape(NT, 128, 3).transpose(1, 0, 2).reshape(128, NT * 3).copy(),
            "cidx16": cidx16,
            "w1": w1.reshape(2, C, C).transpose(1, 0, 2).reshape(C, 2 * C).copy(), "w2": w2, "w3": w3,
            "b1": b1r, "b2": b2r, "b3": b3r,
            "ident": ident, "e16": e16, "sel": sel,
        })

    res = None
    for _attempt in range(3):
        try:
            res = bass_utils.run_bass_kernel_spmd(nc, in_maps, core_ids=list(range(8)))
            break
        except Exception:
            if _attempt == 2:
                raise
            import time as _time
            _time.sleep(2.0)
    out = np.zeros((B, M, OUT), dtype=np.float32)
    knn = np.zeros((B, M, K), dtype=np.int32)
    for core in range(8):
        b = core // 2
        h = core % 2
        r = res.results[core]
        out[b, h * ML:(h + 1) * ML] = r["o_out"]
        knn[b, h * ML:(h + 1) * ML] = r["o_knn"]
    return out, knn


# revision 27
# speedup vs baseline: 1.1064x; 1.1064x over previous
"""Bass/Trainium2 kernel for nn_DynamicRadiusChannelFusion.

Sharding: 8 cores; core j handles batch b=j//2, center half h=j%2 (1024
centers each); points/feats of batch b replicated to its two cores.

knn_idx must match the (neuron-executed, eager per-op) jax reference
bitwise: dist2 = max((a2+b2) - 2*inner, 0), inner = fp32 PE matmul,
selection = stable ascending sort (ties -> lower index). We compute
v2 = min(2*inner - (a2+b2), 0) (bitwise == -dist2), take per-512-chunk
top-8 (nc.vector.max), merge to the top-32 value multiset, then
full-width max_index reproduces the reference tie semantics exactly.
"""
import os
import numpy as np
SKIP_MLP = int(os.environ.get("SKIP_MLP", "0"))

B, N, M, C, OUT, K = 4, 16384, 2048, 128, 256, 32
ML = 1024           # centers per core
NT = ML // 128      # 8 m-tiles
EH = 2048           # edges per MLP half-tile

_cache = {}


def _build():
    import concourse.bacc as bacc
    import concourse.mybir as mybir
    from concourse.tile import TileContext

    f32 = mybir.dt.float32
    u32 = mybir.dt.uint32
    i16 = mybir.dt.int16
    i32 = mybir.dt.int32
    AF = mybir.ActivationFunctionType
    OP = mybir.AluOpType

    nc = bacc.Bacc("TRN2", target_bir_lowering=False, debug=False, num_devices=8)

    t_pointsT = nc.dram_tensor("pointsT", (3, N), f32, kind="ExternalInput")
    t_prows = nc.dram_tensor("prows", (128, N // 128 * 3), f32, kind="ExternalInput")
    t_feats = nc.dram_tensor("feats", (N, C), f32, kind="ExternalInput")
    t_centersT = nc.dram_tensor("centersT", (3, ML), f32, kind="ExternalInput")
    t_crows = nc.dram_tensor("crows", (128, NT * 3), f32, kind="ExternalInput")
    t_cidx = nc.dram_tensor("cidx16", (128, ML // 16), i16, kind="ExternalInput")
    t_w1 = nc.dram_tensor("w1", (C, 2 * C), f32, kind="ExternalInput")
    t_w2 = nc.dram_tensor("w2", (C, C), f32, kind="ExternalInput")
    t_w3 = nc.dram_tensor("w3", (C, OUT), f32, kind="ExternalInput")
    t_b1 = nc.dram_tensor("b1", (128, 1), f32, kind="ExternalInput")
    t_b2 = nc.dram_tensor("b2", (128, 1), f32, kind="ExternalInput")
    t_b3 = nc.dram_tensor("b3", (128, 2), f32, kind="ExternalInput")
    t_ident = nc.dram_tensor("ident", (128, 128), f32, kind="ExternalInput")
    t_e16 = nc.dram_tensor("e16", (16, 128), f32, kind="ExternalInput")
    t_sel = nc.dram_tensor("sel", (64, 4 * 512), f32, kind="ExternalInput")

    o_knn = nc.dram_tensor("o_knn", (ML, K), i32, kind="ExternalOutput")
    o_out = nc.dram_tensor("o_out", (ML, OUT), f32, kind="ExternalOutput")
    d_bsq = nc.dram_tensor("d_bsq", (1, N), f32, kind="Internal")
    o_gidx = nc.dram_tensor("o_gidx", (NT, 128, 256), i16, kind="ExternalOutput")

    with TileContext(nc) as tc:
        with tc.tile_pool(name="cons", bufs=1) as cons, \
             tc.tile_pool(name="big", bufs=1) as big, \
             tc.tile_pool(name="work", bufs=2) as work, \
             tc.tile_pool(name="fat", bufs=1) as fat, \
             tc.tile_pool(name="psA", bufs=2, space="PSUM") as psA, \
             tc.tile_pool(name="psB", bufs=1, space="PSUM") as psB, \
             tc.tile_pool(name="psC", bufs=1, space="PSUM") as psC:

            # ---------------- constants / inputs ----------------
            centersT = cons.tile([3, ML], f32)
            nc.sync.dma_start(centersT, t_centersT.ap())
            crows = cons.tile([128, NT * 3], f32)
            nc.sync.dma_start(crows, t_crows.ap())
            prows = fat.tile([128, N // 128 * 3], f32, tag="fatA")
            nc.sync.dma_start(prows, t_prows.ap())
            w1t = cons.tile([C, 2 * C], f32)
            nc.sync.dma_start(w1t, t_w1.ap())
            w1r = w1t.rearrange("p (h c) -> p h c", h=2)
            w2s = cons.tile([C, C], f32)
            nc.sync.dma_start(w2s, t_w2.ap())
            w3s = cons.tile([C, OUT], f32)
            nc.sync.dma_start(w3s, t_w3.ap())
            b1s = cons.tile([128, 1], f32)
            nc.sync.dma_start(b1s, t_b1.ap())
            b2s = cons.tile([128, 1], f32)
            nc.sync.dma_start(b2s, t_b2.ap())
            b3s = cons.tile([128, 2], f32)
            nc.sync.dma_start(b3s, t_b3.ap())
            ident = cons.tile([128, 128], f32)
            nc.sync.dma_start(ident, t_ident.ap())
            e16 = cons.tile([16, 128], f32)
            nc.sync.dma_start(e16, t_e16.ap())
            sel_t = cons.tile([64, 4 * 512], f32)
            nc.sync.dma_start(sel_t, t_sel.ap())
            sel = sel_t.rearrange("p (q n) -> p q n", q=4)
            cidx = cons.tile([128, ML // 16], i16)
            nc.sync.dma_start(cidx, t_cidx.ap())

            # ---- b_sq bitwise ((x*x + y*y) + z*z) ----
            pr3 = prows.rearrange("p (n c) -> p n c", c=3)
            sq = cons.tile([128, N // 128], f32)
            tq0 = work.tile([128, N // 128], f32, tag="tq0")
            tq1 = work.tile([128, N // 128], f32, tag="tq1")
            nc.vector.tensor_tensor(out=tq0, in0=pr3[:, :, 0], in1=pr3[:, :, 0], op=OP.mult)
            nc.vector.tensor_tensor(out=tq1, in0=pr3[:, :, 1], in1=pr3[:, :, 1], op=OP.mult)
            nc.vector.tensor_tensor(out=tq0, in0=tq0, in1=tq1, op=OP.add)
            nc.vector.tensor_tensor(out=tq1, in0=pr3[:, :, 2], in1=pr3[:, :, 2], op=OP.mult)
            nc.vector.tensor_tensor(out=sq, in0=tq0, in1=tq1, op=OP.add)
            nc.sync.dma_start(d_bsq.ap().rearrange("o (p n) -> p (o n)", p=128), sq)
            ones_col = cons.tile([1, 128], f32)
            nc.vector.memset(ones_col, 1.0)
            bbc = big.tile([128, N], f32)
            for c in range(N // 512):
                bqc = fat.tile([1, 512], f32, tag="fatB")
                nc.sync.dma_start(bqc, d_bsq.ap()[:, c * 512:(c + 1) * 512])
                pb = psB.tile([128, 512], f32, tag="mm512")
                nc.tensor.matmul(pb, ones_col, bqc, start=True, stop=True)
                nc.scalar.activation(bbc[:, c * 512:(c + 1) * 512], pb, AF.Copy,
                                     bias=0.0, scale=1.0)

            # ---- a_sq per center ----
            cr3 = crows.rearrange("p (t c) -> p t c", c=3)
            asq = cons.tile([128, NT], f32)
            ta0 = work.tile([128, NT], f32, tag="ta0")
            ta1 = work.tile([128, NT], f32, tag="ta1")
            nc.vector.tensor_tensor(out=ta0, in0=cr3[:, :, 0], in1=cr3[:, :, 0], op=OP.mult)
            nc.vector.tensor_tensor(out=ta1, in0=cr3[:, :, 1], in1=cr3[:, :, 1], op=OP.mult)
            nc.vector.tensor_tensor(out=ta0, in0=ta0, in1=ta1, op=OP.add)
            nc.vector.tensor_tensor(out=ta1, in0=cr3[:, :, 2], in1=cr3[:, :, 2], op=OP.mult)
            nc.vector.tensor_tensor(out=asq, in0=ta0, in1=ta1, op=OP.add)

            # ---- center feats gather + channel-major + Pa ----
            ctr_g = fat.tile([128, NT, C], f32, tag="fatB")      # m = t*128 + p
            nc.gpsimd.dma_gather(out_ap=ctr_g, in_ap=t_feats.ap(), idxs_ap=cidx,
                                 num_idxs=ML, num_idxs_reg=ML, elem_size=C)
            ctrT = cons.tile([C, ML], f32)
            for t in range(NT):
                pt = psC.tile([128, 128], f32, tag="tr")
                nc.tensor.transpose(pt, ctr_g[:, t, :], ident)
                nc.scalar.activation(ctrT[:, t * 128:(t + 1) * 128], pt, AF.Copy,
                                     bias=0.0, scale=1.0)
            # Pa = w1a^T @ ctrT : (128h, ML)
            Pa = fat.tile([128, ML], f32, tag="fatA")
            for q in range(ML // 512):
                pp = psB.tile([128, 512], f32, tag="mm512")
                nc.tensor.matmul(pp, w1r[:, 0, :], ctrT[:, q * 512:(q + 1) * 512],
                                 start=True, stop=True)
                nc.scalar.activation(Pa[:, q * 512:(q + 1) * 512], pp, AF.Copy,
                                     bias=0.0, scale=1.0)
            # PaT tiles: (m, h) layout per m-tile
            PaT = cons.tile([64, NT * 2, 128], f32)
            for t in range(NT):
                pt = psC.tile([128, 128], f32, tag="tr")
                nc.tensor.transpose(pt, Pa[:, t * 128:(t + 1) * 128], ident)
                nc.scalar.activation(PaT[:, t * 2 + 0, :], pt[0:64, :], AF.Copy, bias=0.0, scale=1.0)
                nc.scalar.activation(PaT[:, t * 2 + 1, :], pt[64:128, :], AF.Copy, bias=0.0, scale=1.0)

            fusedT = cons.tile([C, ML], f32)
            knn_all = cons.tile([128, NT, K], u32)

            # ================= per m-tile =================
            for t in range(NT):
                lhsT = centersT[:, t * 128:(t + 1) * 128]
                v2 = big.tile([128, N], f32, tag="v2")
                for c in range(N // 1024):
                    ptc = work.tile([3, 1024], f32, tag="ptc")
                    nc.sync.dma_start(ptc, t_pointsT.ap()[:, c * 1024:(c + 1) * 1024])
                    pin = psA.tile([128, 1024], f32, tag="inner")
                    for q in range(2):
                        nc.tensor.matmul(pin[:, q * 512:(q + 1) * 512], lhsT,
                                         ptc[:, q * 512:(q + 1) * 512],
                                         start=True, stop=True)
                    sl = slice(c * 1024, (c + 1) * 1024)
                    tab = work.tile([128, 1024], f32, tag="tab")
                    nc.scalar.activation(tab, bbc[:, sl], AF.Relu,
                                         bias=asq[:, t:t + 1], scale=1.0)
                    nc.vector.scalar_tensor_tensor(v2[:, sl], pin, 2.0, tab,
                                                   op0=OP.mult, op1=OP.subtract)
                    nc.vector.tensor_scalar_min(v2[:, sl], v2[:, sl], 0.0)

                cand = work.tile([128, 256], f32, tag="cand")
                for c in range(32):
                    nc.vector.max(out=cand[:, c * 8:(c + 1) * 8],
                                  in_=v2[:, c * 512:(c + 1) * 512])
                scratch = work.tile([128, 256], f32, tag="scratch")
                nc.vector.tensor_copy(scratch, cand)
                fvals = work.tile([128, 32], f32, tag="fvals")
                for r in range(4):
                    nc.vector.max(out=fvals[:, r * 8:(r + 1) * 8], in_=scratch)
                    nc.vector.match_replace(out=scratch,
                                            in_to_replace=fvals[:, r * 8:(r + 1) * 8],
                                            in_values=scratch, imm_value=-3e38)
                knn_t = knn_all[:, t, :]
                for r in range(4):
                    nc.vector.max_index(knn_t[:, r * 8:(r + 1) * 8],
                                        fvals[:, r * 8:(r + 1) * 8], v2)
                    if r < 3:
                        nc.vector.match_replace(out=v2,
                                                in_to_replace=fvals[:, r * 8:(r + 1) * 8],
                                                in_values=v2, imm_value=-3e38)
                nc.sync.dma_start(o_knn.ap()[t * 128:(t + 1) * 128, :],
                                  knn_t.bitcast(i32))

                if SKIP_MLP == 1:
                    continue
                # ---- wrapped idx list (16, 256) -> replicate -> (128,256) --
                knn_f = work.tile([128, K], f32, tag="knnf")
                nc.vector.tensor_copy(knn_f, knn_t)
                ptk_a = psC.tile([16, 128], f32, tag="tr")
                nc.tensor.transpose(ptk_a, knn_f[:, 0:16], ident)
                ptk_b = psC.tile([16, 128], f32, tag="trb")
                nc.tensor.transpose(ptk_b, knn_f[:, 16:32], ident)
                wrap = work.tile([16, 256], f32, tag="wrap")
                wr = wrap.rearrange("p (m h) -> p m h", h=2)
                nc.vector.tensor_copy(wr[:, :, 0], ptk_a)
                nc.vector.tensor_copy(wr[:, :, 1], ptk_b)
                prep = psC.tile([128, 256], f32, tag="tr")
                nc.tensor.matmul(prep, e16, wrap, start=True, stop=True)
                gidx = work.tile([128, 256], i16, tag="gidx")
                nc.vector.tensor_copy(gidx, prep)
                if SKIP_MLP == 3:
                    nc.sync.dma_start(o_gidx.ap()[t], gidx)
                    continue

                # ---- gather neighbor feats + MLP in 2 half-tiles ----------
                for h in range(2):
                    gidx_h = work.tile([128, EH // 16], i16, tag="gidxh")
                    nc.vector.tensor_copy(gidx_h, gidx[:, h * 128:(h + 1) * 128])
                    xg = fat.tile([128, EH // 128, C], f32, tag="fatB")
                    for g4 in range(EH // 1024):
                        nc.gpsimd.dma_gather(
                            out_ap=xg[:, g4 * 8:(g4 + 1) * 8, :], in_ap=t_feats.ap(),
                            idxs_ap=gidx_h[:, g4 * 64:(g4 + 1) * 64],
                            num_idxs=1024, num_idxs_reg=1024, elem_size=C)
                    if SKIP_MLP == 4:
                        continue
                    xnT = fat.tile([C, EH], f32, tag="fatA")
                    for blk in range(EH // 128):
                        pt = psC.tile([128, 128], f32, tag="tr")
                        nc.tensor.transpose(pt, xg[:, blk, :], ident)
                        nc.scalar.activation(xnT[:, blk * 128:(blk + 1) * 128], pt,
                                             AF.Copy, bias=0.0, scale=1.0)
                    hsb = fat.tile([128, EH], f32, tag="fatB")
                    for q in range(EH // 512):
                        ph = psB.tile([128, 512], f32, tag="mm512")
                        nc.tensor.matmul(ph, w1r[:, 1, :],
                                         xnT[:, q * 512:(q + 1) * 512],
                                         start=True, stop=False)
                        nc.tensor.matmul(ph, PaT[:, t * 2 + h, :],
                                         sel[:, q, :],
                                         start=False, stop=True)
                        nc.scalar.activation(hsb[:, q * 512:(q + 1) * 512], ph,
                                             AF.Relu, bias=b1s, scale=1.0)
                    cw = big.tile([128, EH], f32, tag="v2")
                    for q in range(EH // 512):
                        pc = psB.tile([128, 512], f32, tag="mm512")
                        nc.tensor.matmul(pc, w2s,
                                         hsb[:, q * 512:(q + 1) * 512],
                                         start=True, stop=True)
                        nc.scalar.activation(cw[:, q * 512:(q + 1) * 512], pc,
                                             AF.Sigmoid, bias=b2s, scale=1.0)
                    prod = fat.tile([128, EH], f32, tag="fatB")
                    nc.vector.tensor_tensor(out=prod, in0=xnT, in1=cw, op=OP.mult)
                    wsum = work.tile([128, EH // K], f32, tag="wsum")
                    nc.vector.tensor_reduce(
                        wsum, prod.rearrange("p (m k) -> p m k", k=K),
                        axis=mybir.AxisListType.X, op=OP.add)
                    # fused = wsum/K + ctr feats (channel-major)
                    mlo = t * 128 + h * 64
                    nc.vector.scalar_tensor_tensor(
                        fusedT[:, mlo:mlo + 64], wsum, 1.0 / K,
                        ctrT[:, mlo:mlo + 64], op0=OP.mult, op1=OP.add)

            # ---- final layer: out = relu(w3^T @ fusedT + b3) -> (ML, OUT) --
            for t in range(NT if SKIP_MLP == 0 else 0):
                orow = work.tile([128, OUT], f32, tag="orow")
                for j in range(2):
                    po = psB.tile([128, 512], f32, tag="mm512")
                    nc.tensor.matmul(po[:, 0:128], w3s[:, j * 128:(j + 1) * 128],
                                     fusedT[:, t * 128:(t + 1) * 128],
                                     start=True, stop=True)
                    ot = work.tile([128, 128], f32, tag="otmp")
                    nc.scalar.activation(ot, po[:, 0:128],
                                         AF.Relu, bias=b3s[:, j:j + 1], scale=1.0)
                    pt = psC.tile([128, 128], f32, tag="tr")
                    nc.tensor.transpose(pt, ot, ident)
                    nc.scalar.activation(orow[:, j * 128:(j + 1) * 128], pt,
                                         AF.Copy, bias=0.0, scale=1.0)
                nc.sync.dma_start(o_out.ap()[t * 128:(t + 1) * 128, :], orow)

    nc.compile()
    return nc


def kernel(points, feats, center_idx, w1, b1, w2, b2, w3, b3):
    from concourse import bass_utils

    points = np.asarray(points); feats = np.asarray(feats)
    center_idx = np.asarray(center_idx)
    w1 = np.asarray(w1, dtype=np.float32); w2 = np.asarray(w2, dtype=np.float32)
    w3 = np.asarray(w3, dtype=np.float32)
    b1v = np.asarray(b1, dtype=np.float32); b2v = np.asarray(b2, dtype=np.float32)
    b3v = np.asarray(b3, dtype=np.float32)

    if "nc" not in _cache:
        _cache["nc"] = _build()
    nc = _cache["nc"]

    ident = np.eye(128, dtype=np.float32)
    e16 = np.tile(np.eye(16, dtype=np.float32), (1, 8)).reshape(16, 128)
    # e16[k, p] must be 1 iff p % 16 == k
    e16 = np.zeros((16, 128), dtype=np.float32)
    e16[np.arange(128) % 16, np.arange(128)] = 1.0
    sel = np.zeros((64, 4, 512), dtype=np.float32)
    for q in range(4):
        cols = np.arange(512)
        sel[q * 16 + cols // K, q, cols] = 1.0
    sel = sel.reshape(64, 4 * 512)
    b1r = b1v.reshape(128, 1)
    b2r = b2v.reshape(128, 1)
    b3r = b3v.reshape(2, 128).T.copy()   # b3r[p, j] = b3[j*128+p]

    in_maps = []
    for core in range(8):
        b = core // 2
        h = core % 2
        cid = center_idx[b, h * ML:(h + 1) * ML].astype(np.int64)
        P = points[b]
        ctr = P[cid]                                   # (ML, 3)
        # wrapped + replicated int16 idx layout for dma_gather
        cidx16 = np.zeros((128, ML // 16), dtype=np.int16)
        flat = cid.astype(np.int16)
        w = np.zeros((16, ML // 16), dtype=np.int16)
        w[np.arange(ML) % 16, np.arange(ML) // 16] = flat
        cidx16[:] = np.tile(w, (8, 1))
        in_maps.append({
            "pointsT": np.ascontiguousarray(P.T),
            "prows": P.reshape(128, N // 128 * 3).copy(),
            "feats": feats[b].copy(),
            "centersT": np.ascontiguousarray(ctr.T),
            "crows": ctr.reshape(NT, 128, 3).transpose(1, 0, 2).reshape(128, NT * 3).copy(),
            "cidx16": cidx16,
            "w1": w1.reshape(2, C, C).transpose(1, 0, 2).reshape(C, 2 * C).copy(), "w2": w2, "w3": w3,
            "b1": b1r, "b2": b2r, "b3": b3r,
            "ident": ident, "e16": e16, "sel": sel,
        })

    res = None
    for _attempt in range(3):
        try:
            res = bass_utils.run_bass_kernel_spmd(nc, in_maps, core_ids=list(range(8)))
            break
        except Exception:
            if _attempt == 2:
                raise
            import time as _time
            _time.sleep(2.0)
    out = np.zeros((B, M, OUT), dtype=np.float32)
    knn = np.zeros((B, M, K), dtype=np.int32)
    for core in range(8):
        b = core // 2
        h = core % 2
        r = res.results[core]
        out[b, h * ML:(h + 1) * ML] = r["o_out"]
        knn[b, h * ML:(h + 1) * ML] = r["o_knn"]
    return out, knn


# revision 28
# speedup vs baseline: 1.3047x; 1.1793x over previous
"""Bass/Trainium2 kernel for nn_DynamicRadiusChannelFusion.

Sharding: 8 cores; core j handles batch b=j//2, center half h=j%2 (1024
centers each); points/feats of batch b replicated to its two cores.

knn_idx must match the (neuron-executed, eager per-op) jax reference
bitwise: dist2 = max((a2+b2) - 2*inner, 0), inner = fp32 PE matmul,
selection = stable ascending sort (ties -> lower index). We compute
v2 = min(2*inner - (a2+b2), 0) (bitwise == -dist2), take per-512-chunk
top-8 (nc.vector.max), merge to the top-32 value multiset, then
full-width max_index reproduces the reference tie semantics exactly.
"""
import os
import numpy as np
SKIP_MLP = int(os.environ.get("SKIP_MLP", "0"))

B, N, M, C, OUT, K = 4, 16384, 2048, 128, 256, 32
ML = 1024           # centers per core
NT = ML // 128      # 8 m-tiles
EH = 2048           # edges per MLP half-tile

_cache = {}


def _build():
    import concourse.bacc as bacc
    import concourse.mybir as mybir
    from concourse.tile import TileContext

    f32 = mybir.dt.float32
    u32 = mybir.dt.uint32
    i16 = mybir.dt.int16
    i32 = mybir.dt.int32
    AF = mybir.ActivationFunctionType
    OP = mybir.AluOpType

    nc = bacc.Bacc("TRN2", target_bir_lowering=False, debug=False, num_devices=8)

    t_pointsT = nc.dram_tensor("pointsT", (3, N), f32, kind="ExternalInput")
    t_prows = nc.dram_tensor("prows", (128, N // 128 * 3), f32, kind="ExternalInput")
    t_feats = nc.dram_tensor("feats", (N, C), f32, kind="ExternalInput")
    t_centersT = nc.dram_tensor("centersT", (3, ML), f32, kind="ExternalInput")
    t_crows = nc.dram_tensor("crows", (128, NT * 3), f32, kind="ExternalInput")
    t_cidx = nc.dram_tensor("cidx16", (128, ML // 16), i16, kind="ExternalInput")
    t_w1 = nc.dram_tensor("w1", (C, 2 * C), f32, kind="ExternalInput")
    t_w2 = nc.dram_tensor("w2", (C, C), f32, kind="ExternalInput")
    t_w3 = nc.dram_tensor("w3", (C, OUT), f32, kind="ExternalInput")
    t_b1 = nc.dram_tensor("b1", (128, 1), f32, kind="ExternalInput")
    t_b2 = nc.dram_tensor("b2", (128, 1), f32, kind="ExternalInput")
    t_b3 = nc.dram_tensor("b3", (128, 2), f32, kind="ExternalInput")
    t_ident = nc.dram_tensor("ident", (128, 128), f32, kind="ExternalInput")
    t_e16 = nc.dram_tensor("e16", (16, 128), f32, kind="ExternalInput")
    t_sel = nc.dram_tensor("sel", (64, 4 * 512), f32, kind="ExternalInput")
    t_iota = nc.dram_tensor("iota256", (128, 256), f32, kind="ExternalInput")
    t_offs = nc.dram_tensor("offs256", (128, 256), f32, kind="ExternalInput")

    o_knn = nc.dram_tensor("o_knn", (ML, K), i32, kind="ExternalOutput")
    o_out = nc.dram_tensor("o_out", (ML, OUT), f32, kind="ExternalOutput")
    d_bsq = nc.dram_tensor("d_bsq", (1, N), f32, kind="Internal")
    o_gidx = nc.dram_tensor("o_gidx", (NT, 128, 256), i16, kind="ExternalOutput")

    with TileContext(nc) as tc:
        with tc.tile_pool(name="cons", bufs=1) as cons, \
             tc.tile_pool(name="big", bufs=1) as big, \
             tc.tile_pool(name="work", bufs=1) as work, \
             tc.tile_pool(name="fat", bufs=1) as fat, \
             tc.tile_pool(name="psA", bufs=2, space="PSUM") as psA, \
             tc.tile_pool(name="psB", bufs=1, space="PSUM") as psB, \
             tc.tile_pool(name="psC", bufs=1, space="PSUM") as psC:

            # ---------------- constants / inputs ----------------
            centersT = cons.tile([3, ML], f32)
            nc.sync.dma_start(centersT, t_centersT.ap())
            crows = cons.tile([128, NT * 3], f32)
            nc.sync.dma_start(crows, t_crows.ap())
            prows = fat.tile([128, N // 128 * 3], f32, tag="fatA")
            nc.sync.dma_start(prows, t_prows.ap())
            w1t = cons.tile([C, 2 * C], f32)
            nc.sync.dma_start(w1t, t_w1.ap())
            w1r = w1t.rearrange("p (h c) -> p h c", h=2)
            w2s = cons.tile([C, C], f32)
            nc.sync.dma_start(w2s, t_w2.ap())
            w3s = cons.tile([C, OUT], f32)
            nc.sync.dma_start(w3s, t_w3.ap())
            b1s = cons.tile([128, 1], f32)
            nc.sync.dma_start(b1s, t_b1.ap())
            b2s = cons.tile([128, 1], f32)
            nc.sync.dma_start(b2s, t_b2.ap())
            b3s = cons.tile([128, 2], f32)
            nc.sync.dma_start(b3s, t_b3.ap())
            ident = cons.tile([128, 128], f32)
            nc.sync.dma_start(ident, t_ident.ap())
            e16 = cons.tile([16, 128], f32)
            nc.sync.dma_start(e16, t_e16.ap())
            sel_t = cons.tile([64, 4 * 512], f32)
            nc.sync.dma_start(sel_t, t_sel.ap())
            sel = sel_t.rearrange("p (q n) -> p q n", q=4)
            cidx = cons.tile([128, ML // 16], i16)
            nc.sync.dma_start(cidx, t_cidx.ap())
            iota256 = cons.tile([128, 256], f32)
            nc.sync.dma_start(iota256, t_iota.ap())
            offs256 = cons.tile([128, 256], f32)
            nc.sync.dma_start(offs256, t_offs.ap())

            # ---- b_sq bitwise ((x*x + y*y) + z*z) ----
            pr3 = prows.rearrange("p (n c) -> p n c", c=3)
            sq = cons.tile([128, N // 128], f32)
            tq0 = work.tile([128, N // 128], f32, tag="tq0")
            tq1 = work.tile([128, N // 128], f32, tag="tq1")
            nc.vector.tensor_tensor(out=tq0, in0=pr3[:, :, 0], in1=pr3[:, :, 0], op=OP.mult)
            nc.vector.tensor_tensor(out=tq1, in0=pr3[:, :, 1], in1=pr3[:, :, 1], op=OP.mult)
            nc.vector.tensor_tensor(out=tq0, in0=tq0, in1=tq1, op=OP.add)
            nc.vector.tensor_tensor(out=tq1, in0=pr3[:, :, 2], in1=pr3[:, :, 2], op=OP.mult)
            nc.vector.tensor_tensor(out=sq, in0=tq0, in1=tq1, op=OP.add)
            nc.sync.dma_start(d_bsq.ap().rearrange("o (p n) -> p (o n)", p=128), sq)
            ones_col = cons.tile([1, 128], f32)
            nc.vector.memset(ones_col, 1.0)
            bbc = big.tile([128, N], f32)
            for c in range(N // 512):
                bqc = fat.tile([1, 512], f32, tag="fatB")
                nc.sync.dma_start(bqc, d_bsq.ap()[:, c * 512:(c + 1) * 512])
                pb = psB.tile([128, 512], f32, tag="mm512")
                nc.tensor.matmul(pb, ones_col, bqc, start=True, stop=True)
                nc.scalar.activation(bbc[:, c * 512:(c + 1) * 512], pb, AF.Copy,
                                     bias=0.0, scale=1.0)

            # ---- a_sq per center ----
            cr3 = crows.rearrange("p (t c) -> p t c", c=3)
            asq = cons.tile([128, NT], f32)
            ta0 = work.tile([128, NT], f32, tag="ta0")
            ta1 = work.tile([128, NT], f32, tag="ta1")
            nc.vector.tensor_tensor(out=ta0, in0=cr3[:, :, 0], in1=cr3[:, :, 0], op=OP.mult)
            nc.vector.tensor_tensor(out=ta1, in0=cr3[:, :, 1], in1=cr3[:, :, 1], op=OP.mult)
            nc.vector.tensor_tensor(out=ta0, in0=ta0, in1=ta1, op=OP.add)
            nc.vector.tensor_tensor(out=ta1, in0=cr3[:, :, 2], in1=cr3[:, :, 2], op=OP.mult)
            nc.vector.tensor_tensor(out=asq, in0=ta0, in1=ta1, op=OP.add)

            # ---- center feats gather + channel-major + Pa ----
            ctr_g = fat.tile([128, NT, C], f32, tag="fatB")      # m = t*128 + p
            nc.gpsimd.dma_gather(out_ap=ctr_g, in_ap=t_feats.ap(), idxs_ap=cidx,
                                 num_idxs=ML, num_idxs_reg=ML, elem_size=C)
            ctrT = cons.tile([C, ML], f32)
            for t in range(NT):
                pt = psC.tile([128, 128], f32, tag="tr")
                nc.tensor.transpose(pt, ctr_g[:, t, :], ident)
                nc.scalar.activation(ctrT[:, t * 128:(t + 1) * 128], pt, AF.Copy,
                                     bias=0.0, scale=1.0)
            # Pa = w1a^T @ ctrT : (128h, ML)
            Pa = fat.tile([128, ML], f32, tag="fatA")
            for q in range(ML // 512):
                pp = psB.tile([128, 512], f32, tag="mm512")
                nc.tensor.matmul(pp, w1r[:, 0, :], ctrT[:, q * 512:(q + 1) * 512],
                                 start=True, stop=True)
                nc.scalar.activation(Pa[:, q * 512:(q + 1) * 512], pp, AF.Copy,
                                     bias=0.0, scale=1.0)
            # PaT tiles: (m, h) layout per m-tile
            PaT = cons.tile([64, NT * 2, 128], f32)
            for t in range(NT):
                pt = psC.tile([128, 128], f32, tag="tr")
                nc.tensor.transpose(pt, Pa[:, t * 128:(t + 1) * 128], ident)
                nc.scalar.activation(PaT[:, t * 2 + 0, :], pt[0:64, :], AF.Copy, bias=0.0, scale=1.0)
                nc.scalar.activation(PaT[:, t * 2 + 1, :], pt[64:128, :], AF.Copy, bias=0.0, scale=1.0)

            fusedT = cons.tile([C, ML], f32)

            # ================= per m-tile =================
            for t in range(NT):
                lhsT = centersT[:, t * 128:(t + 1) * 128]
                v2 = big.tile([128, N], f32, tag="v2")
                for c in range(N // 1024):
                    ptc = work.tile([3, 1024], f32, tag="ptc")
                    nc.sync.dma_start(ptc, t_pointsT.ap()[:, c * 1024:(c + 1) * 1024])
                    pin = psA.tile([128, 1024], f32, tag="inner")
                    for q in range(2):
                        nc.tensor.matmul(pin[:, q * 512:(q + 1) * 512], lhsT,
                                         ptc[:, q * 512:(q + 1) * 512],
                                         start=True, stop=True)
                    sl = slice(c * 1024, (c + 1) * 1024)
                    tab = work.tile([128, 1024], f32, tag="tab")
                    nc.scalar.activation(tab, bbc[:, sl], AF.Relu,
                                         bias=asq[:, t:t + 1], scale=1.0)
                    nc.vector.scalar_tensor_tensor(v2[:, sl], pin, 2.0, tab,
                                                   op0=OP.mult, op1=OP.subtract)
                    nc.vector.tensor_scalar_min(v2[:, sl], v2[:, sl], 0.0)

                cand = work.tile([128, 256], f32, tag="cand")
                candi = work.tile([128, 256], mybir.dt.uint16, tag="candi")
                for c in range(32):
                    nc.vector.max(out=cand[:, c * 8:(c + 1) * 8],
                                  in_=v2[:, c * 512:(c + 1) * 512])
                    nc.vector.max_index(candi[:, c * 8:(c + 1) * 8],
                                        cand[:, c * 8:(c + 1) * 8],
                                        v2[:, c * 512:(c + 1) * 512])
                cgid = work.tile([128, 256], f32, tag="cgid")
                nc.vector.tensor_copy(cgid, candi)
                nc.vector.tensor_tensor(out=cgid, in0=cgid, in1=offs256, op=OP.add)
                scratch = work.tile([128, 256], f32, tag="scratch")
                nc.vector.tensor_copy(scratch, cand)
                fvals = work.tile([128, 32], f32, tag="fvals")
                for r in range(4):
                    nc.vector.max(out=fvals[:, r * 8:(r + 1) * 8], in_=scratch)
                    nc.vector.match_replace(out=scratch,
                                            in_to_replace=fvals[:, r * 8:(r + 1) * 8],
                                            in_values=scratch, imm_value=-3e38)
                posi = work.tile([128, 32], mybir.dt.uint16, tag="posi")
                for r in range(4):
                    nc.vector.max_index(posi[:, r * 8:(r + 1) * 8],
                                        fvals[:, r * 8:(r + 1) * 8], cand)
                    if r < 3:
                        nc.vector.match_replace(out=cand,
                                                in_to_replace=fvals[:, r * 8:(r + 1) * 8],
                                                in_values=cand, imm_value=-3e38)
                posf = work.tile([128, 32], f32, tag="posf")
                nc.vector.tensor_copy(posf, posi)
                knn_f = work.tile([128, K], f32, tag="knnf")
                ohj = work.tile([128, 256], f32, tag="ohj")
                for j in range(K):
                    nc.vector.tensor_scalar(ohj, iota256, posf[:, j:j + 1], None,
                                            op0=OP.is_equal)
                    nc.vector.tensor_tensor(out=ohj, in0=ohj, in1=cgid, op=OP.mult)
                    nc.vector.tensor_reduce(knn_f[:, j:j + 1], ohj,
                                            axis=mybir.AxisListType.X, op=OP.add)
                knn_i = work.tile([128, K], i32, tag="knni")
                nc.vector.tensor_copy(knn_i, knn_f)
                nc.sync.dma_start(o_knn.ap()[t * 128:(t + 1) * 128, :], knn_i)

                if SKIP_MLP == 1:
                    continue
                # ---- wrapped idx list (16, 256) -> replicate -> (128,256) --
                ptk_a = psC.tile([16, 128], f32, tag="tr")
                nc.tensor.transpose(ptk_a, knn_f[:, 0:16], ident)
                ptk_b = psC.tile([16, 128], f32, tag="trb")
                nc.tensor.transpose(ptk_b, knn_f[:, 16:32], ident)
                wrap = work.tile([16, 256], f32, tag="wrap")
                wr = wrap.rearrange("p (m h) -> p m h", h=2)
                nc.vector.tensor_copy(wr[:, :, 0], ptk_a)
                nc.vector.tensor_copy(wr[:, :, 1], ptk_b)
                prep = psC.tile([128, 256], f32, tag="tr")
                nc.tensor.matmul(prep, e16, wrap, start=True, stop=True)
                gidx = work.tile([128, 256], i16, tag="gidx")
                nc.vector.tensor_copy(gidx, prep)
                if SKIP_MLP == 3:
                    nc.sync.dma_start(o_gidx.ap()[t], gidx)
                    continue

                # ---- gather neighbor feats + MLP in 2 half-tiles ----------
                for h in range(2):
                    gidx_h = work.tile([128, EH // 16], i16, tag="gidxh")
                    nc.vector.tensor_copy(gidx_h, gidx[:, h * 128:(h + 1) * 128])
                    xg = fat.tile([128, EH // 128, C], f32, tag="fatB")
                    for g4 in range(EH // 1024):
                        nc.gpsimd.dma_gather(
                            out_ap=xg[:, g4 * 8:(g4 + 1) * 8, :], in_ap=t_feats.ap(),
                            idxs_ap=gidx_h[:, g4 * 64:(g4 + 1) * 64],
                            num_idxs=1024, num_idxs_reg=1024, elem_size=C)
                    if SKIP_MLP == 4:
                        continue
                    xnT = fat.tile([C, EH], f32, tag="fatA")
                    for blk in range(EH // 128):
                        pt = psC.tile([128, 128], f32, tag="tr")
                        nc.tensor.transpose(pt, xg[:, blk, :], ident)
                        nc.scalar.activation(xnT[:, blk * 128:(blk + 1) * 128], pt,
                                             AF.Copy, bias=0.0, scale=1.0)
                    hsb = fat.tile([128, EH], f32, tag="fatB")
                    for q in range(EH // 512):
                        ph = psB.tile([128, 512], f32, tag="mm512")
                        nc.tensor.matmul(ph, w1r[:, 1, :],
                                         xnT[:, q * 512:(q + 1) * 512],
                                         start=True, stop=False)
                        nc.tensor.matmul(ph, PaT[:, t * 2 + h, :],
                                         sel[:, q, :],
                                         start=False, stop=True)
                        nc.scalar.activation(hsb[:, q * 512:(q + 1) * 512], ph,
                                             AF.Relu, bias=b1s, scale=1.0)
                    cw = big.tile([128, EH], f32, tag="v2")
                    for q in range(EH // 512):
                        pc = psB.tile([128, 512], f32, tag="mm512")
                        nc.tensor.matmul(pc, w2s,
                                         hsb[:, q * 512:(q + 1) * 512],
                                         start=True, stop=True)
                        nc.scalar.activation(cw[:, q * 512:(q + 1) * 512], pc,
                                             AF.Sigmoid, bias=b2s, scale=1.0)
                    prod = fat.tile([128, EH], f32, tag="fatB")
                    nc.vector.tensor_tensor(out=prod, in0=xnT, in1=cw, op=OP.mult)
                    wsum = work.tile([128, EH // K], f32, tag="wsum")
                    nc.vector.tensor_reduce(
                        wsum, prod.rearrange("p (m k) -> p m k", k=K),
                        axis=mybir.AxisListType.X, op=OP.add)
                    # fused = wsum/K + ctr feats (channel-major)
                    mlo = t * 128 + h * 64
                    nc.vector.scalar_tensor_tensor(
                        fusedT[:, mlo:mlo + 64], wsum, 1.0 / K,
                        ctrT[:, mlo:mlo + 64], op0=OP.mult, op1=OP.add)

            # ---- final layer: out = relu(w3^T @ fusedT + b3) -> (ML, OUT) --
            for t in range(NT if SKIP_MLP == 0 else 0):
                orow = work.tile([128, OUT], f32, tag="orow")
                for j in range(2):
                    po = psB.tile([128, 512], f32, tag="mm512")
                    nc.tensor.matmul(po[:, 0:128], w3s[:, j * 128:(j + 1) * 128],
                                     fusedT[:, t * 128:(t + 1) * 128],
                                     start=True, stop=True)
                    ot = work.tile([128, 128], f32, tag="otmp")
                    nc.scalar.activation(ot, po[:, 0:128],
                                         AF.Relu, bias=b3s[:, j:j + 1], scale=1.0)
                    pt = psC.tile([128, 128], f32, tag="tr")
                    nc.tensor.transpose(pt, ot, ident)
                    nc.scalar.activation(orow[:, j * 128:(j + 1) * 128], pt,
                                         AF.Copy, bias=0.0, scale=1.0)
                nc.sync.dma_start(o_out.ap()[t * 128:(t + 1) * 128, :], orow)

    nc.compile()
    return nc


def kernel(points, feats, center_idx, w1, b1, w2, b2, w3, b3):
    from concourse import bass_utils

    points = np.asarray(points); feats = np.asarray(feats)
    center_idx = np.asarray(center_idx)
    w1 = np.asarray(w1, dtype=np.float32); w2 = np.asarray(w2, dtype=np.float32)
    w3 = np.asarray(w3, dtype=np.float32)
    b1v = np.asarray(b1, dtype=np.float32); b2v = np.asarray(b2, dtype=np.float32)
    b3v = np.asarray(b3, dtype=np.float32)

    if "nc" not in _cache:
        _cache["nc"] = _build()
    nc = _cache["nc"]

    ident = np.eye(128, dtype=np.float32)
    e16 = np.tile(np.eye(16, dtype=np.float32), (1, 8)).reshape(16, 128)
    # e16[k, p] must be 1 iff p % 16 == k
    e16 = np.zeros((16, 128), dtype=np.float32)
    e16[np.arange(128) % 16, np.arange(128)] = 1.0
    sel = np.zeros((64, 4, 512), dtype=np.float32)
    for q in range(4):
        cols = np.arange(512)
        sel[q * 16 + cols // K, q, cols] = 1.0
    sel = sel.reshape(64, 4 * 512)
    b1r = b1v.reshape(128, 1)
    b2r = b2v.reshape(128, 1)
    b3r = b3v.reshape(2, 128).T.copy()   # b3r[p, j] = b3[j*128+p]
    iota256 = np.broadcast_to(np.arange(256, dtype=np.float32), (128, 256)).copy()
    offs256 = np.broadcast_to((np.arange(256) // 8 * 512).astype(np.float32), (128, 256)).copy()

    in_maps = []
    for core in range(8):
        b = core // 2
        h = core % 2
        cid = center_idx[b, h * ML:(h + 1) * ML].astype(np.int64)
        P = points[b]
        ctr = P[cid]                                   # (ML, 3)
        # wrapped + replicated int16 idx layout for dma_gather
        cidx16 = np.zeros((128, ML // 16), dtype=np.int16)
        flat = cid.astype(np.int16)
        w = np.zeros((16, ML // 16), dtype=np.int16)
        w[np.arange(ML) % 16, np.arange(ML) // 16] = flat
        cidx16[:] = np.tile(w, (8, 1))
        in_maps.append({
            "pointsT": np.ascontiguousarray(P.T),
            "prows": P.reshape(128, N // 128 * 3).copy(),
            "feats": feats[b].copy(),
            "centersT": np.ascontiguousarray(ctr.T),
            "crows": ctr.reshape(NT, 128, 3).transpose(1, 0, 2).reshape(128, NT * 3).copy(),
            "cidx16": cidx16,
            "w1": w1.reshape(2, C, C).transpose(1, 0, 2).reshape(C, 2 * C).copy(), "w2": w2, "w3": w3,
            "b1": b1r, "b2": b2r, "b3": b3r,
            "ident": ident, "e16": e16, "sel": sel, "iota256": iota256, "offs256": offs256,
        })

    res = None
    for _attempt in range(3):
        try:
            res = bass_utils.run_bass_kernel_spmd(nc, in_maps, core_ids=list(range(8)))
            break
        except Exception:
            if _attempt == 2:
                raise
            import time as _time
            _time.sleep(2.0)
    out = np.zeros((B, M, OUT), dtype=np.float32)
    knn = np.zeros((B, M, K), dtype=np.int32)
    for core in range(8):
        b = core // 2
        h = core % 2
        r = res.results[core]
        out[b, h * ML:(h + 1) * ML] = r["o_out"]
        knn[b, h * ML:(h + 1) * ML] = r["o_knn"]
    return out, knn


# revision 29
# speedup vs baseline: 1.3814x; 1.0588x over previous
"""Bass/Trainium2 kernel for nn_DynamicRadiusChannelFusion.

Sharding: 8 cores; core j handles batch b=j//2, center half h=j%2 (1024
centers each); points/feats of batch b replicated to its two cores.

knn_idx must match the (neuron-executed, eager per-op) jax reference
bitwise: dist2 = max((a2+b2) - 2*inner, 0), inner = fp32 PE matmul,
selection = stable ascending sort (ties -> lower index). We compute
v2 = min(2*inner - (a2+b2), 0) (bitwise == -dist2), take per-512-chunk
top-8 (nc.vector.max), merge to the top-32 value multiset, then
full-width max_index reproduces the reference tie semantics exactly.
"""
import os
import numpy as np
SKIP_MLP = int(os.environ.get("SKIP_MLP", "0"))

B, N, M, C, OUT, K = 4, 16384, 2048, 128, 256, 32
ML = 1024           # centers per core
NT = ML // 128      # 8 m-tiles
EH = 2048           # edges per MLP half-tile

_cache = {}


def _build():
    import concourse.bacc as bacc
    import concourse.mybir as mybir
    from concourse.tile import TileContext

    f32 = mybir.dt.float32
    u32 = mybir.dt.uint32
    i16 = mybir.dt.int16
    i32 = mybir.dt.int32
    AF = mybir.ActivationFunctionType
    OP = mybir.AluOpType

    nc = bacc.Bacc("TRN2", target_bir_lowering=False, debug=False, num_devices=8)

    t_pointsT = nc.dram_tensor("pointsT", (3, N), f32, kind="ExternalInput")
    t_prows = nc.dram_tensor("prows", (128, N // 128 * 3), f32, kind="ExternalInput")
    t_feats = nc.dram_tensor("feats", (N, C), f32, kind="ExternalInput")
    t_centersT = nc.dram_tensor("centersT", (3, ML), f32, kind="ExternalInput")
    t_crows = nc.dram_tensor("crows", (128, NT * 3), f32, kind="ExternalInput")
    t_cidx = nc.dram_tensor("cidx16", (128, ML // 16), i16, kind="ExternalInput")
    t_w1 = nc.dram_tensor("w1", (C, 2 * C), f32, kind="ExternalInput")
    t_w2 = nc.dram_tensor("w2", (C, C), f32, kind="ExternalInput")
    t_w3 = nc.dram_tensor("w3", (C, OUT), f32, kind="ExternalInput")
    t_b1 = nc.dram_tensor("b1", (128, 1), f32, kind="ExternalInput")
    t_b2 = nc.dram_tensor("b2", (128, 1), f32, kind="ExternalInput")
    t_b3 = nc.dram_tensor("b3", (128, 2), f32, kind="ExternalInput")
    t_ident = nc.dram_tensor("ident", (128, 128), f32, kind="ExternalInput")
    t_e16 = nc.dram_tensor("e16", (16, 128), f32, kind="ExternalInput")
    t_sel = nc.dram_tensor("sel", (64, 4 * 512), f32, kind="ExternalInput")
    t_iota = nc.dram_tensor("iota256", (128, 256), f32, kind="ExternalInput")
    t_offs = nc.dram_tensor("offs256", (128, 256), f32, kind="ExternalInput")

    o_knn = nc.dram_tensor("o_knn", (ML, K), i32, kind="ExternalOutput")
    o_out = nc.dram_tensor("o_out", (ML, OUT), f32, kind="ExternalOutput")
    d_bsq = nc.dram_tensor("d_bsq", (1, N), f32, kind="Internal")
    o_gidx = nc.dram_tensor("o_gidx", (NT, 128, 256), i16, kind="ExternalOutput")

    with TileContext(nc) as tc:
        with tc.tile_pool(name="cons", bufs=1) as cons, \
             tc.tile_pool(name="big", bufs=1) as big, \
             tc.tile_pool(name="work", bufs=1) as work, \
             tc.tile_pool(name="fat", bufs=1) as fat, \
             tc.tile_pool(name="psA", bufs=2, space="PSUM") as psA, \
             tc.tile_pool(name="psB", bufs=2, space="PSUM") as psB, \
             tc.tile_pool(name="psC", bufs=1, space="PSUM") as psC:

            # ---------------- constants / inputs ----------------
            centersT = cons.tile([3, ML], f32)
            nc.sync.dma_start(centersT, t_centersT.ap())
            crows = cons.tile([128, NT * 3], f32)
            nc.sync.dma_start(crows, t_crows.ap())
            prows = fat.tile([128, N // 128 * 3], f32, tag="fatA")
            nc.sync.dma_start(prows, t_prows.ap())
            w1t = cons.tile([C, 2 * C], f32)
            nc.sync.dma_start(w1t, t_w1.ap())
            w1r = w1t.rearrange("p (h c) -> p h c", h=2)
            w2s = cons.tile([C, C], f32)
            nc.sync.dma_start(w2s, t_w2.ap())
            w3s = cons.tile([C, OUT], f32)
            nc.sync.dma_start(w3s, t_w3.ap())
            b1s = cons.tile([128, 1], f32)
            nc.sync.dma_start(b1s, t_b1.ap())
            b2s = cons.tile([128, 1], f32)
            nc.sync.dma_start(b2s, t_b2.ap())
            b3s = cons.tile([128, 2], f32)
            nc.sync.dma_start(b3s, t_b3.ap())
            ident = cons.tile([128, 128], f32)
            nc.sync.dma_start(ident, t_ident.ap())
            e16 = cons.tile([16, 128], f32)
            nc.sync.dma_start(e16, t_e16.ap())
            sel_t = cons.tile([64, 4 * 512], f32)
            nc.sync.dma_start(sel_t, t_sel.ap())
            sel = sel_t.rearrange("p (q n) -> p q n", q=4)
            cidx = cons.tile([128, ML // 16], i16)
            nc.sync.dma_start(cidx, t_cidx.ap())
            iota256 = cons.tile([128, 256], f32)
            nc.sync.dma_start(iota256, t_iota.ap())
            offs256 = cons.tile([128, 256], f32)
            nc.sync.dma_start(offs256, t_offs.ap())

            # ---- b_sq bitwise ((x*x + y*y) + z*z) ----
            pr3 = prows.rearrange("p (n c) -> p n c", c=3)
            sq = cons.tile([128, N // 128], f32)
            tq0 = work.tile([128, N // 128], f32, tag="tq0")
            tq1 = work.tile([128, N // 128], f32, tag="tq1")
            nc.vector.tensor_tensor(out=tq0, in0=pr3[:, :, 0], in1=pr3[:, :, 0], op=OP.mult)
            nc.vector.tensor_tensor(out=tq1, in0=pr3[:, :, 1], in1=pr3[:, :, 1], op=OP.mult)
            nc.vector.tensor_tensor(out=tq0, in0=tq0, in1=tq1, op=OP.add)
            nc.vector.tensor_tensor(out=tq1, in0=pr3[:, :, 2], in1=pr3[:, :, 2], op=OP.mult)
            nc.vector.tensor_tensor(out=sq, in0=tq0, in1=tq1, op=OP.add)
            nc.sync.dma_start(d_bsq.ap().rearrange("o (p n) -> p (o n)", p=128), sq)
            ones_col = cons.tile([1, 128], f32)
            nc.vector.memset(ones_col, 1.0)
            bbc = big.tile([128, N], f32)
            for c in range(N // 512):
                bqc = fat.tile([1, 512], f32, tag="fatB")
                nc.sync.dma_start(bqc, d_bsq.ap()[:, c * 512:(c + 1) * 512])
                pb = psB.tile([128, 512], f32, tag="mm512")
                nc.tensor.matmul(pb, ones_col, bqc, start=True, stop=True)
                nc.scalar.activation(bbc[:, c * 512:(c + 1) * 512], pb, AF.Copy,
                                     bias=0.0, scale=1.0)

            # ---- a_sq per center ----
            cr3 = crows.rearrange("p (t c) -> p t c", c=3)
            asq = cons.tile([128, NT], f32)
            ta0 = work.tile([128, NT], f32, tag="ta0")
            ta1 = work.tile([128, NT], f32, tag="ta1")
            nc.vector.tensor_tensor(out=ta0, in0=cr3[:, :, 0], in1=cr3[:, :, 0], op=OP.mult)
            nc.vector.tensor_tensor(out=ta1, in0=cr3[:, :, 1], in1=cr3[:, :, 1], op=OP.mult)
            nc.vector.tensor_tensor(out=ta0, in0=ta0, in1=ta1, op=OP.add)
            nc.vector.tensor_tensor(out=ta1, in0=cr3[:, :, 2], in1=cr3[:, :, 2], op=OP.mult)
            nc.vector.tensor_tensor(out=asq, in0=ta0, in1=ta1, op=OP.add)

            # ---- center feats gather + channel-major + Pa ----
            ctr_g = fat.tile([128, NT, C], f32, tag="fatB")      # m = t*128 + p
            nc.gpsimd.dma_gather(out_ap=ctr_g, in_ap=t_feats.ap(), idxs_ap=cidx,
                                 num_idxs=ML, num_idxs_reg=ML, elem_size=C)
            ctrT = cons.tile([C, ML], f32)
            for t in range(NT):
                pt = psC.tile([128, 128], f32, tag="tr")
                nc.tensor.transpose(pt, ctr_g[:, t, :], ident)
                nc.scalar.activation(ctrT[:, t * 128:(t + 1) * 128], pt, AF.Copy,
                                     bias=0.0, scale=1.0)
            # Pa = w1a^T @ ctrT : (128h, ML)
            Pa = fat.tile([128, ML], f32, tag="fatA")
            for q in range(ML // 512):
                pp = psB.tile([128, 512], f32, tag="mm512")
                nc.tensor.matmul(pp, w1r[:, 0, :], ctrT[:, q * 512:(q + 1) * 512],
                                 start=True, stop=True)
                nc.scalar.activation(Pa[:, q * 512:(q + 1) * 512], pp, AF.Copy,
                                     bias=0.0, scale=1.0)
            # PaT tiles: (m, h) layout per m-tile
            PaT = cons.tile([64, NT * 2, 128], f32)
            for t in range(NT):
                pt = psC.tile([128, 128], f32, tag="tr")
                nc.tensor.transpose(pt, Pa[:, t * 128:(t + 1) * 128], ident)
                nc.scalar.activation(PaT[:, t * 2 + 0, :], pt[0:64, :], AF.Copy, bias=0.0, scale=1.0)
                nc.scalar.activation(PaT[:, t * 2 + 1, :], pt[64:128, :], AF.Copy, bias=0.0, scale=1.0)

            fusedT = cons.tile([C, ML], f32)

            # ================= per m-tile =================
            for t in range(NT):
                lhsT = centersT[:, t * 128:(t + 1) * 128]
                v2 = big.tile([128, N], f32, tag="v2")
                for c in range(N // 1024):
                    ptc = work.tile([3, 1024], f32, tag="ptc")
                    nc.sync.dma_start(ptc, t_pointsT.ap()[:, c * 1024:(c + 1) * 1024])
                    pin = psA.tile([128, 1024], f32, tag="inner")
                    for q in range(2):
                        nc.tensor.matmul(pin[:, q * 512:(q + 1) * 512], lhsT,
                                         ptc[:, q * 512:(q + 1) * 512],
                                         start=True, stop=True)
                    sl = slice(c * 1024, (c + 1) * 1024)
                    tab = work.tile([128, 1024], f32, tag="tab")
                    nc.scalar.activation(tab, bbc[:, sl], AF.Relu,
                                         bias=asq[:, t:t + 1], scale=1.0)
                    nc.vector.scalar_tensor_tensor(v2[:, sl], pin, 2.0, tab,
                                                   op0=OP.mult, op1=OP.subtract)
                    nc.vector.tensor_scalar_min(v2[:, sl], v2[:, sl], 0.0)

                cand = work.tile([128, 256], f32, tag="cand")
                candi = work.tile([128, 256], mybir.dt.uint16, tag="candi")
                for c in range(32):
                    nc.vector.max(out=cand[:, c * 8:(c + 1) * 8],
                                  in_=v2[:, c * 512:(c + 1) * 512])
                    nc.vector.max_index(candi[:, c * 8:(c + 1) * 8],
                                        cand[:, c * 8:(c + 1) * 8],
                                        v2[:, c * 512:(c + 1) * 512])
                cgid = work.tile([128, 256], f32, tag="cgid")
                nc.vector.tensor_copy(cgid, candi)
                nc.vector.tensor_tensor(out=cgid, in0=cgid, in1=offs256, op=OP.add)
                scratch = work.tile([128, 256], f32, tag="scratch")
                nc.vector.tensor_copy(scratch, cand)
                fvals = work.tile([128, 32], f32, tag="fvals")
                for r in range(4):
                    nc.vector.max(out=fvals[:, r * 8:(r + 1) * 8], in_=scratch)
                    nc.vector.match_replace(out=scratch,
                                            in_to_replace=fvals[:, r * 8:(r + 1) * 8],
                                            in_values=scratch, imm_value=-3e38)
                posi = work.tile([128, 32], mybir.dt.uint16, tag="posi")
                for r in range(4):
                    nc.vector.max_index(posi[:, r * 8:(r + 1) * 8],
                                        fvals[:, r * 8:(r + 1) * 8], cand)
                    if r < 3:
                        nc.vector.match_replace(out=cand,
                                                in_to_replace=fvals[:, r * 8:(r + 1) * 8],
                                                in_values=cand, imm_value=-3e38)
                posf = work.tile([128, 32], f32, tag="posf")
                nc.vector.tensor_copy(posf, posi)
                knn_f = work.tile([128, K], f32, tag="knnf")
                ohj = work.tile([128, 256], f32, tag="ohj")
                for j in range(K):
                    nc.vector.tensor_scalar(ohj, iota256, posf[:, j:j + 1], None,
                                            op0=OP.is_equal)
                    nc.vector.tensor_tensor(out=ohj, in0=ohj, in1=cgid, op=OP.mult)
                    nc.vector.tensor_reduce(knn_f[:, j:j + 1], ohj,
                                            axis=mybir.AxisListType.X, op=OP.add)
                knn_i = work.tile([128, K], i32, tag="knni")
                nc.vector.tensor_copy(knn_i, knn_f)
                nc.sync.dma_start(o_knn.ap()[t * 128:(t + 1) * 128, :], knn_i)

                if SKIP_MLP == 1:
                    continue
                # ---- wrapped idx list (16, 256) -> replicate -> (128,256) --
                ptk_a = psC.tile([16, 128], f32, tag="tr")
                nc.tensor.transpose(ptk_a, knn_f[:, 0:16], ident)
                ptk_b = psC.tile([16, 128], f32, tag="trb")
                nc.tensor.transpose(ptk_b, knn_f[:, 16:32], ident)
                wrap = work.tile([16, 256], f32, tag="wrap")
                wr = wrap.rearrange("p (m h) -> p m h", h=2)
                nc.vector.tensor_copy(wr[:, :, 0], ptk_a)
                nc.vector.tensor_copy(wr[:, :, 1], ptk_b)
                prep = psC.tile([128, 256], f32, tag="tr")
                nc.tensor.matmul(prep, e16, wrap, start=True, stop=True)
                gidx = work.tile([128, 256], i16, tag="gidx")
                nc.vector.tensor_copy(gidx, prep)
                if SKIP_MLP == 3:
                    nc.sync.dma_start(o_gidx.ap()[t], gidx)
                    continue

                # ---- gather neighbor feats + MLP in 2 half-tiles ----------
                for h in range(2):
                    gidx_h = work.tile([128, EH // 16], i16, tag="gidxh")
                    nc.vector.tensor_copy(gidx_h, gidx[:, h * 128:(h + 1) * 128])
                    xg = fat.tile([128, EH // 128, C], f32, tag="fatB")
                    for g4 in range(EH // 1024):
                        nc.gpsimd.dma_gather(
                            out_ap=xg[:, g4 * 8:(g4 + 1) * 8, :], in_ap=t_feats.ap(),
                            idxs_ap=gidx_h[:, g4 * 64:(g4 + 1) * 64],
                            num_idxs=1024, num_idxs_reg=1024, elem_size=C)
                    if SKIP_MLP == 4:
                        continue
                    xnT = fat.tile([C, EH], f32, tag="fatA")
                    for blk in range(EH // 128):
                        pt = psC.tile([128, 128], f32, tag="tr")
                        nc.tensor.transpose(pt, xg[:, blk, :], ident)
                        nc.scalar.activation(xnT[:, blk * 128:(blk + 1) * 128], pt,
                                             AF.Copy, bias=0.0, scale=1.0)
                    hsb = fat.tile([128, EH], f32, tag="fatB")
                    for q in range(EH // 512):
                        ph = psB.tile([128, 512], f32, tag="mm512")
                        nc.tensor.matmul(ph, w1r[:, 1, :],
                                         xnT[:, q * 512:(q + 1) * 512],
                                         start=True, stop=False)
                        nc.tensor.matmul(ph, PaT[:, t * 2 + h, :],
                                         sel[:, q, :],
                                         start=False, stop=True)
                        nc.scalar.activation(hsb[:, q * 512:(q + 1) * 512], ph,
                                             AF.Relu, bias=b1s, scale=1.0)
                    cw = big.tile([128, EH], f32, tag="v2")
                    for q in range(EH // 512):
                        pc = psB.tile([128, 512], f32, tag="mm512")
                        nc.tensor.matmul(pc, w2s,
                                         hsb[:, q * 512:(q + 1) * 512],
                                         start=True, stop=True)
                        nc.scalar.activation(cw[:, q * 512:(q + 1) * 512], pc,
                                             AF.Sigmoid, bias=b2s, scale=1.0)
                    prod = fat.tile([128, EH], f32, tag="fatB")
                    nc.vector.tensor_tensor(out=prod, in0=xnT, in1=cw, op=OP.mult)
                    wsum = work.tile([128, EH // K], f32, tag="wsum")
                    nc.vector.tensor_reduce(
                        wsum, prod.rearrange("p (m k) -> p m k", k=K),
                        axis=mybir.AxisListType.X, op=OP.add)
                    # fused = wsum/K + ctr feats (channel-major)
                    mlo = t * 128 + h * 64
                    nc.vector.scalar_tensor_tensor(
                        fusedT[:, mlo:mlo + 64], wsum, 1.0 / K,
                        ctrT[:, mlo:mlo + 64], op0=OP.mult, op1=OP.add)

            # ---- final layer: out = relu(w3^T @ fusedT + b3) -> (ML, OUT) --
            for t in range(NT if SKIP_MLP == 0 else 0):
                orow = work.tile([128, OUT], f32, tag="orow")
                for j in range(2):
                    po = psB.tile([128, 512], f32, tag="mm512")
                    nc.tensor.matmul(po[:, 0:128], w3s[:, j * 128:(j + 1) * 128],
                                     fusedT[:, t * 128:(t + 1) * 128],
                                     start=True, stop=True)
                    ot = work.tile([128, 128], f32, tag="otmp")
                    nc.scalar.activation(ot, po[:, 0:128],
                                         AF.Relu, bias=b3s[:, j:j + 1], scale=1.0)
                    pt = psC.tile([128, 128], f32, tag="tr")
                    nc.tensor.transpose(pt, ot, ident)
                    nc.scalar.activation(orow[:, j * 128:(j + 1) * 128], pt,
                                         AF.Copy, bias=0.0, scale=1.0)
                nc.sync.dma_start(o_out.ap()[t * 128:(t + 1) * 128, :], orow)

    nc.compile()
    return nc


def kernel(points, feats, center_idx, w1, b1, w2, b2, w3, b3):
    from concourse import bass_utils

    points = np.asarray(points); feats = np.asarray(feats)
    center_idx = np.asarray(center_idx)
    w1 = np.asarray(w1, dtype=np.float32); w2 = np.asarray(w2, dtype=np.float32)
    w3 = np.asarray(w3, dtype=np.float32)
    b1v = np.asarray(b1, dtype=np.float32); b2v = np.asarray(b2, dtype=np.float32)
    b3v = np.asarray(b3, dtype=np.float32)

    if "nc" not in _cache:
        _cache["nc"] = _build()
    nc = _cache["nc"]

    ident = np.eye(128, dtype=np.float32)
    e16 = np.tile(np.eye(16, dtype=np.float32), (1, 8)).reshape(16, 128)
    # e16[k, p] must be 1 iff p % 16 == k
    e16 = np.zeros((16, 128), dtype=np.float32)
    e16[np.arange(128) % 16, np.arange(128)] = 1.0
    sel = np.zeros((64, 4, 512), dtype=np.float32)
    for q in range(4):
        cols = np.arange(512)
        sel[q * 16 + cols // K, q, cols] = 1.0
    sel = sel.reshape(64, 4 * 512)
    b1r = b1v.reshape(128, 1)
    b2r = b2v.reshape(128, 1)
    b3r = b3v.reshape(2, 128).T.copy()   # b3r[p, j] = b3[j*128+p]
    iota256 = np.broadcast_to(np.arange(256, dtype=np.float32), (128, 256)).copy()
    offs256 = np.broadcast_to((np.arange(256) // 8 * 512).astype(np.float32), (128, 256)).copy()

    in_maps = []
    for core in range(8):
        b = core // 2
        h = core % 2
        cid = center_idx[b, h * ML:(h + 1) * ML].astype(np.int64)
        P = points[b]
        ctr = P[cid]                                   # (ML, 3)
        # wrapped + replicated int16 idx layout for dma_gather
        cidx16 = np.zeros((128, ML // 16), dtype=np.int16)
        flat = cid.astype(np.int16)
        w = np.zeros((16, ML // 16), dtype=np.int16)
        w[np.arange(ML) % 16, np.arange(ML) // 16] = flat
        cidx16[:] = np.tile(w, (8, 1))
        in_maps.append({
            "pointsT": np.ascontiguousarray(P.T),
            "prows": P.reshape(128, N // 128 * 3).copy(),
            "feats": feats[b].copy(),
            "centersT": np.ascontiguousarray(ctr.T),
            "crows": ctr.reshape(NT, 128, 3).transpose(1, 0, 2).reshape(128, NT * 3).copy(),
            "cidx16": cidx16,
            "w1": w1.reshape(2, C, C).transpose(1, 0, 2).reshape(C, 2 * C).copy(), "w2": w2, "w3": w3,
            "b1": b1r, "b2": b2r, "b3": b3r,
            "ident": ident, "e16": e16, "sel": sel, "iota256": iota256, "offs256": offs256,
        })

    res = None
    for _attempt in range(3):
        try:
            res = bass_utils.run_bass_kernel_spmd(nc, in_maps, core_ids=list(range(8)))
            break
        except Exception:
            if _attempt == 2:
                raise
            import time as _time
            _time.sleep(2.0)
    out = np.zeros((B, M, OUT), dtype=np.float32)
    knn = np.zeros((B, M, K), dtype=np.int32)
    for core in range(8):
        b = core // 2
        h = core % 2
        r = res.results[core]
        out[b, h * ML:(h + 1) * ML] = r["o_out"]
        knn[b, h * ML:(h + 1) * ML] = r["o_knn"]
    return out, knn


# revision 30
# speedup vs baseline: 1.4498x; 1.0495x over previous
"""Bass/Trainium2 kernel for nn_DynamicRadiusChannelFusion.

Sharding: 8 cores; core j handles batch b=j//2, center half h=j%2 (1024
centers each); points/feats of batch b replicated to its two cores.

knn_idx must match the (neuron-executed, eager per-op) jax reference
bitwise: dist2 = max((a2+b2) - 2*inner, 0), inner = fp32 PE matmul,
selection = stable ascending sort (ties -> lower index). We compute
v2 = min(2*inner - (a2+b2), 0) (bitwise == -dist2), take per-512-chunk
top-8 (nc.vector.max), merge to the top-32 value multiset, then
full-width max_index reproduces the reference tie semantics exactly.
"""
import os
import numpy as np
SKIP_MLP = int(os.environ.get("SKIP_MLP", "0"))

B, N, M, C, OUT, K = 4, 16384, 2048, 128, 256, 32
ML = 1024           # centers per core
NT = ML // 128      # 8 m-tiles
EH = 2048           # edges per MLP half-tile

_cache = {}


def _build():
    import concourse.bacc as bacc
    import concourse.mybir as mybir
    from concourse.tile import TileContext

    f32 = mybir.dt.float32
    u32 = mybir.dt.uint32
    i16 = mybir.dt.int16
    i32 = mybir.dt.int32
    AF = mybir.ActivationFunctionType
    OP = mybir.AluOpType

    nc = bacc.Bacc("TRN2", target_bir_lowering=False, debug=False, num_devices=8)

    t_pointsT = nc.dram_tensor("pointsT", (3, N), f32, kind="ExternalInput")
    t_prows = nc.dram_tensor("prows", (128, N // 128 * 3), f32, kind="ExternalInput")
    t_feats = nc.dram_tensor("feats", (N, C), f32, kind="ExternalInput")
    t_centersT = nc.dram_tensor("centersT", (3, ML), f32, kind="ExternalInput")
    t_crows = nc.dram_tensor("crows", (128, NT * 3), f32, kind="ExternalInput")
    t_cidx = nc.dram_tensor("cidx16", (128, ML // 16), i16, kind="ExternalInput")
    t_w1 = nc.dram_tensor("w1", (C, 2 * C), f32, kind="ExternalInput")
    t_w2 = nc.dram_tensor("w2", (C, C), f32, kind="ExternalInput")
    t_w3 = nc.dram_tensor("w3", (C, OUT), f32, kind="ExternalInput")
    t_b1 = nc.dram_tensor("b1", (128, 1), f32, kind="ExternalInput")
    t_b2 = nc.dram_tensor("b2", (128, 1), f32, kind="ExternalInput")
    t_b3 = nc.dram_tensor("b3", (128, 2), f32, kind="ExternalInput")
    t_ident = nc.dram_tensor("ident", (128, 128), f32, kind="ExternalInput")
    t_e16 = nc.dram_tensor("e16", (16, 128), f32, kind="ExternalInput")
    t_sel = nc.dram_tensor("sel", (64, 4 * 512), f32, kind="ExternalInput")
    t_iota = nc.dram_tensor("iota256", (128, 256), f32, kind="ExternalInput")
    t_offs = nc.dram_tensor("offs256", (128, 256), f32, kind="ExternalInput")

    o_knn = nc.dram_tensor("o_knn", (ML, K), i32, kind="ExternalOutput")
    o_out = nc.dram_tensor("o_out", (ML, OUT), f32, kind="ExternalOutput")
    d_bsq = nc.dram_tensor("d_bsq", (1, N), f32, kind="Internal")
    o_gidx = nc.dram_tensor("o_gidx", (NT, 128, 256), i16, kind="ExternalOutput")

    with TileContext(nc) as tc:
        with tc.tile_pool(name="cons", bufs=1) as cons, \
             tc.tile_pool(name="big", bufs=1) as big, \
             tc.tile_pool(name="work", bufs=1) as work, \
             tc.tile_pool(name="fat", bufs=1) as fat, \
             tc.tile_pool(name="psA", bufs=2, space="PSUM") as psA, \
             tc.tile_pool(name="psB", bufs=2, space="PSUM") as psB, \
             tc.tile_pool(name="psC", bufs=1, space="PSUM") as psC:

            # ---------------- constants / inputs ----------------
            centersT = cons.tile([3, ML], f32)
            nc.sync.dma_start(centersT, t_centersT.ap())
            crows = cons.tile([128, NT * 3], f32)
            nc.sync.dma_start(crows, t_crows.ap())
            prows = fat.tile([128, N // 128 * 3], f32, tag="fatA")
            nc.sync.dma_start(prows, t_prows.ap())
            w1t = cons.tile([C, 2 * C], f32)
            nc.sync.dma_start(w1t, t_w1.ap())
            w1r = w1t.rearrange("p (h c) -> p h c", h=2)
            w2s = cons.tile([C, C], f32)
            nc.sync.dma_start(w2s, t_w2.ap())
            w3s = cons.tile([C, OUT], f32)
            nc.sync.dma_start(w3s, t_w3.ap())
            b1s = cons.tile([128, 1], f32)
            nc.sync.dma_start(b1s, t_b1.ap())
            b2s = cons.tile([128, 1], f32)
            nc.sync.dma_start(b2s, t_b2.ap())
            b3s = cons.tile([128, 2], f32)
            nc.sync.dma_start(b3s, t_b3.ap())
            ident = cons.tile([128, 128], f32)
            nc.sync.dma_start(ident, t_ident.ap())
            e16 = cons.tile([16, 128], f32)
            nc.sync.dma_start(e16, t_e16.ap())
            sel_t = cons.tile([64, 4 * 512], f32)
            nc.sync.dma_start(sel_t, t_sel.ap())
            sel = sel_t.rearrange("p (q n) -> p q n", q=4)
            cidx = cons.tile([128, ML // 16], i16)
            nc.sync.dma_start(cidx, t_cidx.ap())
            iota256 = cons.tile([128, 256], f32)
            nc.sync.dma_start(iota256, t_iota.ap())
            offs256 = cons.tile([128, 256], f32)
            nc.sync.dma_start(offs256, t_offs.ap())

            # ---- b_sq bitwise ((x*x + y*y) + z*z) ----
            pr3 = prows.rearrange("p (n c) -> p n c", c=3)
            sq = cons.tile([128, N // 128], f32)
            tq0 = work.tile([128, N // 128], f32, tag="tq0")
            tq1 = work.tile([128, N // 128], f32, tag="tq1")
            nc.vector.tensor_tensor(out=tq0, in0=pr3[:, :, 0], in1=pr3[:, :, 0], op=OP.mult)
            nc.vector.tensor_tensor(out=tq1, in0=pr3[:, :, 1], in1=pr3[:, :, 1], op=OP.mult)
            nc.vector.tensor_tensor(out=tq0, in0=tq0, in1=tq1, op=OP.add)
            nc.vector.tensor_tensor(out=tq1, in0=pr3[:, :, 2], in1=pr3[:, :, 2], op=OP.mult)
            nc.vector.tensor_tensor(out=sq, in0=tq0, in1=tq1, op=OP.add)
            nc.sync.dma_start(d_bsq.ap().rearrange("o (p n) -> p (o n)", p=128), sq)
            ones_col = cons.tile([1, 128], f32)
            nc.vector.memset(ones_col, 1.0)
            bbc = big.tile([128, N], f32)
            for c in range(N // 512):
                bqc = fat.tile([1, 512], f32, tag="fatB")
                nc.sync.dma_start(bqc, d_bsq.ap()[:, c * 512:(c + 1) * 512])
                pb = psB.tile([128, 512], f32, tag="mm512")
                nc.tensor.matmul(pb, ones_col, bqc, start=True, stop=True)
                nc.scalar.activation(bbc[:, c * 512:(c + 1) * 512], pb, AF.Copy,
                                     bias=0.0, scale=1.0)

            # ---- a_sq per center ----
            cr3 = crows.rearrange("p (t c) -> p t c", c=3)
            asq = cons.tile([128, NT], f32)
            ta0 = work.tile([128, NT], f32, tag="ta0")
            ta1 = work.tile([128, NT], f32, tag="ta1")
            nc.vector.tensor_tensor(out=ta0, in0=cr3[:, :, 0], in1=cr3[:, :, 0], op=OP.mult)
            nc.vector.tensor_tensor(out=ta1, in0=cr3[:, :, 1], in1=cr3[:, :, 1], op=OP.mult)
            nc.vector.tensor_tensor(out=ta0, in0=ta0, in1=ta1, op=OP.add)
            nc.vector.tensor_tensor(out=ta1, in0=cr3[:, :, 2], in1=cr3[:, :, 2], op=OP.mult)
            nc.vector.tensor_tensor(out=asq, in0=ta0, in1=ta1, op=OP.add)

            # ---- center feats gather + channel-major + Pa ----
            ctr_g = fat.tile([128, NT, C], f32, tag="fatB")      # m = t*128 + p
            nc.gpsimd.dma_gather(out_ap=ctr_g, in_ap=t_feats.ap(), idxs_ap=cidx,
                                 num_idxs=ML, num_idxs_reg=ML, elem_size=C)
            ctrT = cons.tile([C, ML], f32)
            for t in range(NT):
                pt = psC.tile([128, 128], f32, tag="tr")
                nc.tensor.transpose(pt, ctr_g[:, t, :], ident)
                nc.scalar.activation(ctrT[:, t * 128:(t + 1) * 128], pt, AF.Copy,
                                     bias=0.0, scale=1.0)
            # Pa = w1a^T @ ctrT : (128h, ML)
            Pa = fat.tile([128, ML], f32, tag="fatA")
            for q in range(ML // 512):
                pp = psB.tile([128, 512], f32, tag="mm512")
                nc.tensor.matmul(pp, w1r[:, 0, :], ctrT[:, q * 512:(q + 1) * 512],
                                 start=True, stop=True)
                nc.scalar.activation(Pa[:, q * 512:(q + 1) * 512], pp, AF.Copy,
                                     bias=0.0, scale=1.0)
            # PaT tiles: (m, h) layout per m-tile
            PaT = cons.tile([64, NT * 2, 128], f32)
            for t in range(NT):
                pt = psC.tile([128, 128], f32, tag="tr")
                nc.tensor.transpose(pt, Pa[:, t * 128:(t + 1) * 128], ident)
                nc.scalar.activation(PaT[:, t * 2 + 0, :], pt[0:64, :], AF.Copy, bias=0.0, scale=1.0)
                nc.scalar.activation(PaT[:, t * 2 + 1, :], pt[64:128, :], AF.Copy, bias=0.0, scale=1.0)

            fusedT = cons.tile([C, ML], f32)

            # ================= per m-tile =================
            for t in range(NT):
                lhsT = centersT[:, t * 128:(t + 1) * 128]
                v2 = big.tile([128, N], f32, tag="v2")
                for c in range(N // 1024):
                    ptc = work.tile([3, 1024], f32, tag="ptc")
                    nc.sync.dma_start(ptc, t_pointsT.ap()[:, c * 1024:(c + 1) * 1024])
                    pin = psA.tile([128, 1024], f32, tag="inner")
                    for q in range(2):
                        nc.tensor.matmul(pin[:, q * 512:(q + 1) * 512], lhsT,
                                         ptc[:, q * 512:(q + 1) * 512],
                                         start=True, stop=True)
                    sl = slice(c * 1024, (c + 1) * 1024)
                    tab = work.tile([128, 1024], f32, tag="tab")
                    nc.scalar.activation(tab, bbc[:, sl], AF.Relu,
                                         bias=asq[:, t:t + 1], scale=1.0)
                    nc.vector.scalar_tensor_tensor(v2[:, sl], pin, 2.0, tab,
                                                   op0=OP.mult, op1=OP.subtract)
                    nc.vector.tensor_scalar_min(v2[:, sl], v2[:, sl], 0.0)

                cand = work.tile([128, 256], f32, tag="cand")
                candi = work.tile([128, 256], mybir.dt.uint16, tag="candi")
                for c in range(32):
                    nc.vector.max(out=cand[:, c * 8:(c + 1) * 8],
                                  in_=v2[:, c * 512:(c + 1) * 512])
                    nc.vector.max_index(candi[:, c * 8:(c + 1) * 8],
                                        cand[:, c * 8:(c + 1) * 8],
                                        v2[:, c * 512:(c + 1) * 512])
                cgid = work.tile([128, 256], f32, tag="cgid")
                nc.vector.tensor_copy(cgid, candi)
                nc.vector.tensor_tensor(out=cgid, in0=cgid, in1=offs256, op=OP.add)
                scratch = work.tile([128, 256], f32, tag="scratch")
                nc.vector.tensor_copy(scratch, cand)
                fvals = work.tile([128, 32], f32, tag="fvals")
                for r in range(4):
                    nc.vector.max(out=fvals[:, r * 8:(r + 1) * 8], in_=scratch)
                    nc.vector.match_replace(out=scratch,
                                            in_to_replace=fvals[:, r * 8:(r + 1) * 8],
                                            in_values=scratch, imm_value=-3e38)
                posi = work.tile([128, 32], mybir.dt.uint16, tag="posi")
                for r in range(4):
                    nc.vector.max_index(posi[:, r * 8:(r + 1) * 8],
                                        fvals[:, r * 8:(r + 1) * 8], cand)
                    if r < 3:
                        nc.vector.match_replace(out=cand,
                                                in_to_replace=fvals[:, r * 8:(r + 1) * 8],
                                                in_values=cand, imm_value=-3e38)
                posf = work.tile([128, 32], f32, tag="posf")
                nc.vector.tensor_copy(posf, posi)
                knn_f = work.tile([128, K], f32, tag="knnf")
                ohj = work.tile([128, 256], f32, tag="ohj")
                for j in range(K):
                    nc.vector.scalar_tensor_tensor(ohj, iota256, posf[:, j:j + 1],
                                                   cgid, op0=OP.is_equal, op1=OP.mult)
                    nc.vector.tensor_reduce(knn_f[:, j:j + 1], ohj,
                                            axis=mybir.AxisListType.X, op=OP.add)
                knn_i = work.tile([128, K], i32, tag="knni")
                nc.vector.tensor_copy(knn_i, knn_f)
                nc.sync.dma_start(o_knn.ap()[t * 128:(t + 1) * 128, :], knn_i)

                if SKIP_MLP == 1:
                    continue
                # ---- wrapped idx list (16, 256) -> replicate -> (128,256) --
                ptk_a = psC.tile([16, 128], f32, tag="tr")
                nc.tensor.transpose(ptk_a, knn_f[:, 0:16], ident)
                ptk_b = psC.tile([16, 128], f32, tag="trb")
                nc.tensor.transpose(ptk_b, knn_f[:, 16:32], ident)
                wrap = work.tile([16, 256], f32, tag="wrap")
                wr = wrap.rearrange("p (m h) -> p m h", h=2)
                nc.vector.tensor_copy(wr[:, :, 0], ptk_a)
                nc.vector.tensor_copy(wr[:, :, 1], ptk_b)
                prep = psC.tile([128, 256], f32, tag="tr")
                nc.tensor.matmul(prep, e16, wrap, start=True, stop=True)
                gidx = work.tile([128, 256], i16, tag="gidx")
                nc.vector.tensor_copy(gidx, prep)
                if SKIP_MLP == 3:
                    nc.sync.dma_start(o_gidx.ap()[t], gidx)
                    continue

                # ---- gather neighbor feats + MLP in 2 half-tiles ----------
                for h in range(2):
                    gidx_h = work.tile([128, EH // 16], i16, tag="gidxh")
                    nc.vector.tensor_copy(gidx_h, gidx[:, h * 128:(h + 1) * 128])
                    xg = fat.tile([128, EH // 128, C], f32, tag="fatB")
                    for g4 in range(EH // 1024):
                        nc.gpsimd.dma_gather(
                            out_ap=xg[:, g4 * 8:(g4 + 1) * 8, :], in_ap=t_feats.ap(),
                            idxs_ap=gidx_h[:, g4 * 64:(g4 + 1) * 64],
                            num_idxs=1024, num_idxs_reg=1024, elem_size=C)
                    if SKIP_MLP == 4:
                        continue
                    xnT = fat.tile([C, EH], f32, tag="fatA")
                    for blk in range(EH // 128):
                        pt = psC.tile([128, 128], f32, tag="tr")
                        nc.tensor.transpose(pt, xg[:, blk, :], ident)
                        nc.scalar.activation(xnT[:, blk * 128:(blk + 1) * 128], pt,
                                             AF.Copy, bias=0.0, scale=1.0)
                    hsb = fat.tile([128, EH], f32, tag="fatB")
                    for q in range(EH // 512):
                        ph = psB.tile([128, 512], f32, tag="mm512")
                        nc.tensor.matmul(ph, w1r[:, 1, :],
                                         xnT[:, q * 512:(q + 1) * 512],
                                         start=True, stop=False)
                        nc.tensor.matmul(ph, PaT[:, t * 2 + h, :],
                                         sel[:, q, :],
                                         start=False, stop=True)
                        nc.scalar.activation(hsb[:, q * 512:(q + 1) * 512], ph,
                                             AF.Relu, bias=b1s, scale=1.0)
                    cw = big.tile([128, EH], f32, tag="v2")
                    for q in range(EH // 512):
                        pc = psB.tile([128, 512], f32, tag="mm512")
                        nc.tensor.matmul(pc, w2s,
                                         hsb[:, q * 512:(q + 1) * 512],
                                         start=True, stop=True)
                        nc.scalar.activation(cw[:, q * 512:(q + 1) * 512], pc,
                                             AF.Sigmoid, bias=b2s, scale=1.0)
                    prod = fat.tile([128, EH], f32, tag="fatB")
                    nc.vector.tensor_tensor(out=prod, in0=xnT, in1=cw, op=OP.mult)
                    wsum = work.tile([128, EH // K], f32, tag="wsum")
                    nc.vector.tensor_reduce(
                        wsum, prod.rearrange("p (m k) -> p m k", k=K),
                        axis=mybir.AxisListType.X, op=OP.add)
                    # fused = wsum/K + ctr feats (channel-major)
                    mlo = t * 128 + h * 64
                    nc.vector.scalar_tensor_tensor(
                        fusedT[:, mlo:mlo + 64], wsum, 1.0 / K,
                        ctrT[:, mlo:mlo + 64], op0=OP.mult, op1=OP.add)

            # ---- final layer: out = relu(w3^T @ fusedT + b3) -> (ML, OUT) --
            for t in range(NT if SKIP_MLP == 0 else 0):
                orow = work.tile([128, OUT], f32, tag="orow")
                for j in range(2):
                    po = psB.tile([128, 512], f32, tag="mm512")
                    nc.tensor.matmul(po[:, 0:128], w3s[:, j * 128:(j + 1) * 128],
                                     fusedT[:, t * 128:(t + 1) * 128],
                                     start=True, stop=True)
                    ot = work.tile([128, 128], f32, tag="otmp")
                    nc.scalar.activation(ot, po[:, 0:128],
                                         AF.Relu, bias=b3s[:, j:j + 1], scale=1.0)
                    pt = psC.tile([128, 128], f32, tag="tr")
                    nc.tensor.transpose(pt, ot, ident)
                    nc.scalar.activation(orow[:, j * 128:(j + 1) * 128], pt,
                                         AF.Copy, bias=0.0, scale=1.0)
                nc.sync.dma_start(o_out.ap()[t * 128:(t + 1) * 128, :], orow)

    nc.compile()
    return nc


def kernel(points, feats, center_idx, w1, b1, w2, b2, w3, b3):
    from concourse import bass_utils

    points = np.asarray(points); feats = np.asarray(feats)
    center_idx = np.asarray(center_idx)
    w1 = np.asarray(w1, dtype=np.float32); w2 = np.asarray(w2, dtype=np.float32)
    w3 = np.asarray(w3, dtype=np.float32)
    b1v = np.asarray(b1, dtype=np.float32); b2v = np.asarray(b2, dtype=np.float32)
    b3v = np.asarray(b3, dtype=np.float32)

    if "nc" not in _cache:
        _cache["nc"] = _build()
    nc = _cache["nc"]

    ident = np.eye(128, dtype=np.float32)
    e16 = np.tile(np.eye(16, dtype=np.float32), (1, 8)).reshape(16, 128)
    # e16[k, p] must be 1 iff p % 16 == k
    e16 = np.zeros((16, 128), dtype=np.float32)
    e16[np.arange(128) % 16, np.arange(128)] = 1.0
    sel = np.zeros((64, 4, 512), dtype=np.float32)
    for q in range(4):
        cols = np.arange(512)
        sel[q * 16 + cols // K, q, cols] = 1.0
    sel = sel.reshape(64, 4 * 512)
    b1r = b1v.reshape(128, 1)
    b2r = b2v.reshape(128, 1)
    b3r = b3v.reshape(2, 128).T.copy()   # b3r[p, j] = b3[j*128+p]
    iota256 = np.broadcast_to(np.arange(256, dtype=np.float32), (128, 256)).copy()
    offs256 = np.broadcast_to((np.arange(256) // 8 * 512).astype(np.float32), (128, 256)).copy()

    in_maps = []
    for core in range(8):
        b = core // 2
        h = core % 2
        cid = center_idx[b, h * ML:(h + 1) * ML].astype(np.int64)
        P = points[b]
        ctr = P[cid]                                   # (ML, 3)
        # wrapped + replicated int16 idx layout for dma_gather
        cidx16 = np.zeros((128, ML // 16), dtype=np.int16)
        flat = cid.astype(np.int16)
        w = np.zeros((16, ML // 16), dtype=np.int16)
        w[np.arange(ML) % 16, np.arange(ML) // 16] = flat
        cidx16[:] = np.tile(w, (8, 1))
        in_maps.append({
            "pointsT": np.ascontiguousarray(P.T),
            "prows": P.reshape(128, N // 128 * 3).copy(),
            "feats": feats[b].copy(),
            "centersT": np.ascontiguousarray(ctr.T),
            "crows": ctr.reshape(NT, 128, 3).transpose(1, 0, 2).reshape(128, NT * 3).copy(),
            "cidx16": cidx16,
            "w1": w1.reshape(2, C, C).transpose(1, 0, 2).reshape(C, 2 * C).copy(), "w2": w2, "w3": w3,
            "b1": b1r, "b2": b2r, "b3": b3r,
            "ident": ident, "e16": e16, "sel": sel, "iota256": iota256, "offs256": offs256,
        })

    res = None
    for _attempt in range(3):
        try:
            res = bass_utils.run_bass_kernel_spmd(nc, in_maps, core_ids=list(range(8)))
            break
        except Exception:
            if _attempt == 2:
                raise
            import time as _time
            _time.sleep(2.0)
    out = np.zeros((B, M, OUT), dtype=np.float32)
    knn = np.zeros((B, M, K), dtype=np.int32)
    for core in range(8):
        b = core // 2
        h = core % 2
        r = res.results[core]
        out[b, h * ML:(h + 1) * ML] = r["o_out"]
        knn[b, h * ML:(h + 1) * ML] = r["o_knn"]
    return out, knn
